# revision 12
# baseline (speedup 1.0000x reference)
"""Trainium2 Bass kernel for Llama-style GQA attention (B=1, S=2048, D=4096,
32 Q heads / 8 KV heads, head_dim 128, RoPE, additive mask, causal-aware).

Sharding: 8-way tensor-parallel over heads. Core c computes Q heads 4c..4c+3
and KV head c end-to-end; the host sums the 8 partial [S, D] outputs (the
all-reduce of the row-parallel wo).

v4 strategy (over v3): the PE runs ONLY real matmuls -- the softmax
denominator and its broadcast are moved off the PE entirely:
  - Fused per-group sweep: K, V and the 4 Q heads accumulate in one k-loop
    (6 matmuls per k-tile, 3 PSUM pairs: kv + 2 q-pairs).  x is streamed
    once per group and each chunk's buffer frees early (smooth prefetch).
  - Post-sweep, ACT stages the 3 PSUM pairs to SBUF bf16 fast (pair slots
    recycle in ~1us); RoPE runs later from the copies (all-bf16, 2x DVE
    rate) interleaved into the NEXT PE-heavy phase where the DVE is idle.
    V dma-transposes ride the idle sync ring.
  - Softmax sum: DVE adds each exp-pair (bf16), GPSIMD accumulates the
    pair-sums (fp32) and partition_all_reduce broadcasts the key-sum to
    all partitions; DVE reciprocal + in-place normalize of the context.
    The context is staged out of PSUM unnormalized (DVE copy) so the PSUM
    bank ring never waits on the slow gpsimd chain; reciprocal+multiply
    are emitted one phase later (deadline: pass C reads ctx much later).
  - Diagonal mask tiles at offsets 256/384 are computed at half width
    (columns below the offset are fully masked); their exp(mask) patterns
    make full-width reads exact where needed.
  - Schedule: sweep(G+1) is emitted before attention(G) so RoPE/copies
    have a whole phase of slack; pass C (wo) gets 3-deep PSUM pipelining.
"""

import math
import os
import numpy as np

os.environ.setdefault("NEURON_RT_RESET_CORES", "1")

P = 128          # SBUF partitions / head_dim / tile edge
S = 2048         # sequence length
D = 4096         # model dim
HD = 128         # head dim
N_HEADS = 32
N_KV = 8
N_CORES = 8
NH_LOC = N_HEADS // N_CORES   # 4 local Q heads
SG = 512         # score/free-dim group width (one PSUM bank of fp32)
NG = S // SG     # 4 q-position groups
KT = D // P      # 32 contraction tiles for projections
NSK = S // P     # 16 key tiles

_CACHE = {}


def _classify_mask(mask):
    """Classify each [P, SG] block of mask.T into skip / plain / masked."""
    mt = np.ascontiguousarray(mask.T.astype(np.float32))
    patterns = []
    pat_idx = {}
    sk_lists = []
    for G in range(NG):
        lst = []
        for m in range(NSK):
            blk = mt[m * P:(m + 1) * P, G * SG:(G + 1) * SG]
            if np.all(np.isneginf(blk)):
                continue
            if np.all(blk == 0.0):
                lst.append((m, None))
                continue
            with np.errstate(over="ignore"):
                pat = np.exp(blk).astype(np.float32)
            key = pat.tobytes()
            if key not in pat_idx:
                pat_idx[key] = len(patterns)
                patterns.append(pat)
            lst.append((m, pat_idx[key]))
        sk_lists.append(lst)
    return sk_lists, patterns


def _diag_off(pat_np, thresh=256):
    """Column offset below which a pattern block is entirely zero (0 if the
    leading-zero span is < thresh; offsets are quantized to {0, 256})."""
    colmax = pat_np.max(axis=0)
    nz = np.nonzero(colmax)[0]
    first = int(nz[0]) if len(nz) else pat_np.shape[1]
    return 256 if first >= thresh else 0


def _build_program(sk_lists, n_pat, pat_offs):
    import concourse.tile as tile
    from concourse import bacc, mybir
    from concourse.bass_isa import ReduceOp
    from contextlib import ExitStack

    f32 = mybir.dt.float32
    bf = mybir.dt.bfloat16
    Exp = mybir.ActivationFunctionType.Exp

    nc = bacc.Bacc()
    xt_d = nc.dram_tensor("xt", [P, NG * KT * SG], bf, kind="ExternalInput")
    wq_d = nc.dram_tensor("wq", [P, KT * NH_LOC * HD], bf, kind="ExternalInput")
    wk_d = nc.dram_tensor("wk", [P, KT * HD], bf, kind="ExternalInput")
    wv_d = nc.dram_tensor("wv", [P, KT * HD], bf, kind="ExternalInput")
    wo_d = nc.dram_tensor("wo", [P, (D // SG) * NH_LOC * SG], bf,
                          kind="ExternalInput")
    # [cos;sin] in cols [0:S], [sin;cos] in cols [S:2S] -- both partition
    # layouts, because DVE tensor_tensor requires equal base partitions
    # when both inputs are in SBUF
    cs_d = nc.dram_tensor("cs", [P, 2 * S], bf, kind="ExternalInput")
    mb_d = None
    if n_pat:
        mb_d = nc.dram_tensor("mb", [n_pat, P, SG], bf, kind="ExternalInput")
    out_d = nc.dram_tensor("out", [S, D], bf, kind="ExternalOutput")

    XCH = 4 * SG     # xT DMA chunk: 4 k-tiles, 4KB per partition line

    with ExitStack() as ctx:
        tc = ctx.enter_context(tile.TileContext(nc))
        consts = ctx.enter_context(tc.tile_pool(name="consts", bufs=1))
        kv = ctx.enter_context(tc.tile_pool(name="kv", bufs=1))
        xp = ctx.enter_context(tc.tile_pool(name="xp", bufs=8))
        qp = ctx.enter_context(tc.tile_pool(name="qp", bufs=8))
        rp = ctx.enter_context(tc.tile_pool(name="rp", bufs=4))
        ep = ctx.enter_context(tc.tile_pool(name="ep", bufs=4))
        sp = ctx.enter_context(tc.tile_pool(name="sp", bufs=4))
        ap_ = ctx.enter_context(tc.tile_pool(name="ap", bufs=8))
        cp = ctx.enter_context(tc.tile_pool(name="cp", bufs=4))
        ps = ctx.enter_context(tc.tile_pool(name="ps", bufs=8, space="PSUM"))

        # resident weights / constants on the scalar ring; the xT stream and
        # V transposes own the sync ring.
        wk_sb = consts.tile([P, KT * HD], bf)
        wv_sb = consts.tile([P, KT * HD], bf)
        wq_sb = consts.tile([P, KT * NH_LOC * HD], bf)
        # leads so the first k-tiles' matmuls start as early as possible
        nc.scalar.dma_start(wk_sb[:, 0:2 * HD], wk_d[:, 0:2 * HD])
        nc.scalar.dma_start(wv_sb[:, 0:2 * HD], wv_d[:, 0:2 * HD])
        wqt = NH_LOC * HD  # per-k-tile wq block
        nc.scalar.dma_start(wq_sb[:, 0:2 * wqt], wq_d[:, 0:2 * wqt])
        half = KT * HD // 2
        nc.scalar.dma_start(wk_sb[:, 2 * HD:half], wk_d[:, 2 * HD:half])
        nc.scalar.dma_start(wv_sb[:, 2 * HD:half], wv_d[:, 2 * HD:half])
        nc.scalar.dma_start(wk_sb[:, half:], wk_d[:, half:])
        nc.scalar.dma_start(wv_sb[:, half:], wv_d[:, half:])
        qqt = KT * NH_LOC * HD // 8
        for i in range(8):
            lo = max(i * qqt, 2 * wqt)
            if lo < (i + 1) * qqt:
                nc.scalar.dma_start(wq_sb[:, lo:(i + 1) * qqt],
                                    wq_d[:, lo:(i + 1) * qqt])
        cs_sb = consts.tile([P, 2 * S], bf)
        nc.scalar.dma_start(cs_sb[:, 0:S], cs_d[:, 0:S])
        nc.scalar.dma_start(cs_sb[:, S:2 * S], cs_d[:, S:2 * S])
        mb_sb = None
        if n_pat:
            mb_sb = consts.tile([P, n_pat * SG], bf, name="mb_sb")
        wo_sb = consts.tile([P, (D // SG) * NH_LOC * SG], bf)

        def load_mb():
            for i in range(n_pat):
                nc.scalar.dma_start(mb_sb[:, i * SG:(i + 1) * SG], mb_d[i])

        def load_wo():
            for i in range(8):
                nc.scalar.dma_start(wo_sb[:, i * qqt:(i + 1) * qqt],
                                    wo_d[:, i * qqt:(i + 1) * qqt])

        # full-sequence KV + context accumulators
        kT_sb = kv.tile([P, S], bf)                  # [head_dim', s]
        v_sb = kv.tile([P, S], bf)                   # [s%P, (s//P)*HD + hd]
        ctx_sb = kv.tile([P, NH_LOC * S], bf)        # [hd, h*S + sq]

        def stream_x(G):
            """DMA the 8 xT chunks of group G; returns per-k slices."""
            slices = []
            for c2 in range(KT * SG // XCH):
                xw = xp.tile([P, XCH], bf, tag="xt", bufs=8, name="xt")
                blk = G * KT * SG + c2 * XCH
                if G == 0 and c2 == 0:
                    nc.sync.dma_start(xw[:, 0:SG], xt_d[:, blk:blk + SG])
                    nc.sync.dma_start(xw[:, SG:XCH],
                                      xt_d[:, blk + SG:blk + XCH])
                else:
                    nc.sync.dma_start(xw[:], xt_d[:, blk:blk + XCH])
                for j in range(XCH // SG):
                    slices.append(xw[:, j * SG:(j + 1) * SG])
            return slices

        def rope(src, dr, di, G):
            cos0 = cs_sb[0:64, G * SG:(G + 1) * SG]
            sin64 = cs_sb[64:128, G * SG:(G + 1) * SG]
            sin0 = cs_sb[0:64, S + G * SG:S + (G + 1) * SG]
            cos64 = cs_sb[64:128, S + G * SG:S + (G + 1) * SG]
            ta = rp.tile([64, SG], bf, tag="ropeA", bufs=2)
            tb = rp.tile([64, SG], bf, tag="ropeB", bufs=2)
            nc.vector.tensor_mul(ta[:], src[0:64, :], cos0)
            nc.vector.tensor_mul(tb[:], src[64:128, :], sin64)
            nc.vector.tensor_sub(dr, ta[:], tb[:])
            tc2 = rp.tile([64, SG], bf, tag="ropeA", bufs=2)
            td = rp.tile([64, SG], bf, tag="ropeB", bufs=2)
            nc.vector.tensor_mul(tc2[:], src[0:64, :], sin0)
            nc.vector.tensor_mul(td[:], src[64:128, :], cos64)
            nc.vector.tensor_add(di, tc2[:], td[:])

        qts = {}
        rope_q = []      # DVE: RoPE of the latest sweep (run in next phase)
        fin_q = []       # DVE: reciprocal + in-place ctx normalize

        def flush(queue, n=None):
            todo = queue[:n] if n is not None else queue[:]
            del queue[:len(todo)]
            for t in todo:
                t()

        def sweep(G, xs):
            """Fused K/V/Q projections for s-group G: 6 matmuls per k-tile."""
            pkv = ps.tile([P, 2 * SG], f32, tag="pair", bufs=3, name="pkv")
            pq1 = ps.tile([P, 2 * SG], f32, tag="pair", bufs=3, name="pq1")
            pq2 = ps.tile([P, 2 * SG], f32, tag="pair", bufs=3, name="pq2")
            dsts = [pkv[:, 0:SG], pkv[:, SG:2 * SG],
                    pq1[:, 0:SG], pq1[:, SG:2 * SG],
                    pq2[:, 0:SG], pq2[:, SG:2 * SG]]
            for k in range(KT):
                st, sp_ = (k == 0), (k == KT - 1)
                nc.tensor.matmul(dsts[0], wk_sb[:, k * HD:(k + 1) * HD],
                                 xs[k], start=st, stop=sp_)
                nc.tensor.matmul(dsts[1], wv_sb[:, k * HD:(k + 1) * HD],
                                 xs[k], start=st, stop=sp_)
                for l in range(NH_LOC):
                    nc.tensor.matmul(
                        dsts[2 + l],
                        wq_sb[:, (k * NH_LOC + l) * HD:(k * NH_LOC + l + 1) * HD],
                        xs[k], start=st, stop=sp_)
                # previous sweep's deferred DVE work rides this PE-heavy loop
                if k % 6 == 5 and rope_q:
                    flush(rope_q, 1)
                if k % 8 == 7 and fin_q:
                    flush(fin_q, 1)
            # stage PSUM -> SBUF bf16 promptly on ACT: pair slots recycle in
            # ~1us each so the next phase's stp allocations never stall long
            kvc = sp.tile([P, 2 * SG], bf, tag="swcp", bufs=3, name="kvc")
            q1c = sp.tile([P, 2 * SG], bf, tag="swcp", bufs=3, name="q1c")
            q2c = sp.tile([P, 2 * SG], bf, tag="swcp", bufs=3, name="q2c")
            nc.scalar.copy(kvc[:], pkv[:])
            nc.scalar.copy(q1c[:], pq1[:])
            nc.scalar.copy(q2c[:], pq2[:])
            # V transposes on the idle sync ring (deadline: attention(G))
            for j in range(SG // P):
                nc.sync.dma_start_transpose(
                    v_sb[:, (G * 4 + j) * HD:(G * 4 + j + 1) * HD],
                    kvc[:, SG + j * P:SG + (j + 1) * P])
            # RoPE from the SBUF copies, deferred into the next PE phase
            def rope_k(G=G, kvc=kvc):
                gsl = slice(G * SG, (G + 1) * SG)
                rope(kvc[:, 0:SG], kT_sb[0:64, gsl], kT_sb[64:128, gsl], G)
            rope_q.append(rope_k)
            for i, (src, lo) in enumerate(((q1c, 0), (q1c, SG),
                                           (q2c, 0), (q2c, SG))):
                dst = qp.tile([P, SG], bf, tag="qT", bufs=8, name="qT")
                qts[(G, i)] = dst
                def rope_qh(src=src, lo=lo, dst=dst, G=G):
                    rope(src[:, lo:lo + SG], dst[0:64, :], dst[64:128, :], G)
                rope_q.append(rope_qh)

        def pass_b(G, drain_ropes=False):
            # order: diagonal tiles first (d0 full-width carries start=True),
            # then full tiles; the LAST tile is full-width (carries stop) --
            # for G0 that's the offset-384 diag with its dead region zeroed.
            diag = [e for e in sk_lists[G] if e[1] is not None]
            plain = [e for e in sk_lists[G] if e[1] is None]
            diag.sort(key=lambda e: e[0])
            # (m, pat, col_off)
            tiles = [(m, pat, pat_offs[pat]) for m, pat in diag] + \
                    [(m, pat, 0) for m, pat in plain]
            n_sk = len(tiles)
            npair = (n_sk + 1) // 2
            g0 = not plain
            for h in range(NH_LOC):
                cacc = ps.tile([P, SG], f32, tag="bank", bufs=2, name="cacc")
                acc = ap_.tile([P, SG], f32, tag="acc", bufs=2)
                # bufs=4: the reciprocal reading sums(h) is emitted a phase
                # later, so all 4 heads' sums must stay live
                sums = ap_.tile([P, SG], f32, tag="sums", bufs=4)
                nfull = [0]
                part_ts = []

                def emit_pair(p):
                    w = 2 if 2 * p + 1 < n_sk else 1
                    pr = tiles[2 * p:2 * p + w]
                    stp = ps.tile([P, 2 * SG], f32, tag="pair", bufs=3,
                                  name="stp")
                    for j, (m, pat, off) in enumerate(pr):
                        nc.tensor.matmul(stp[:, j * SG + off:(j + 1) * SG],
                                         kT_sb[:, m * P:(m + 1) * P],
                                         qts[(G, h)][:, off:SG],
                                         start=True, stop=True)
                    ex = ep.tile([P, 2 * SG], bf, tag="ex", bufs=4)
                    offs = [t[2] for t in pr]
                    if w == 2 and offs[0] == 0 and offs[1] == 0:
                        nc.scalar.activation(ex[:, 0:2 * SG], stp[:, 0:2 * SG],
                                             Exp)
                    else:
                        for j, (m, pat, off) in enumerate(pr):
                            nc.scalar.activation(
                                ex[:, j * SG + off:(j + 1) * SG],
                                stp[:, j * SG + off:(j + 1) * SG], Exp)
                    if g0 and p == npair - 1:
                        # last G0 tile runs its PV full-width to carry the
                        # stop flag; zero its dead region
                        nc.vector.memset(ex[:, SG:SG + offs[1]], 0.0)
                    for j, (m, pat, off) in enumerate(pr):
                        if pat is not None:
                            nc.vector.tensor_mul(
                                ex[:, j * SG + off:(j + 1) * SG],
                                ex[:, j * SG + off:(j + 1) * SG],
                                mb_sb[:, pat * SG + off:(pat + 1) * SG])
                    # softmax-denominator pair-sum on the DVE (bf16); fp32
                    # accumulation chain runs on gpsimd -- no PE involvement
                    t = ap_.tile([P, SG], bf, tag="tp", bufs=8)
                    moff = max(offs)
                    if w == 2:
                        nc.vector.tensor_add(t[:, moff:SG],
                                             ex[:, moff:SG],
                                             ex[:, SG + moff:2 * SG])
                    else:
                        nc.vector.tensor_copy(t[:, moff:SG], ex[:, moff:SG])
                    if moff:
                        part_ts.append((t, moff))
                    else:
                        nfull[0] += 1
                        if nfull[0] == 1:
                            nc.gpsimd.tensor_copy(acc[:], t[:])
                        else:
                            nc.gpsimd.tensor_add(acc[:], acc[:], t[:])
                    return ex

                exq = [emit_pair(p) for p in range(min(3, npair))]
                for p in range(npair):
                    ex = exq[p]
                    w = 2 if 2 * p + 1 < n_sk else 1
                    for j, (m, pat, off) in enumerate(tiles[2 * p:2 * p + w]):
                        idx = 2 * p + j
                        if g0 and idx == n_sk - 1:
                            off = 0      # full-width PV carries the stop
                        nc.tensor.matmul(cacc[:, off:SG],
                                         v_sb[:, m * HD:(m + 1) * HD],
                                         ex[:, j * SG + off:(j + 1) * SG],
                                         start=(idx == 0), stop=(idx == n_sk - 1))
                    if p + 3 < npair:
                        exq.append(emit_pair(p + 3))
                # stage the unnormalized context out of PSUM so the bank
                # ring never waits on the gpsimd chain
                gsl = slice(h * S + G * SG, h * S + (G + 1) * SG)
                nc.vector.tensor_copy(ctx_sb[:, gsl], cacc[:])
                for t, moff in part_ts:
                    nc.gpsimd.tensor_add(acc[:, moff:SG], acc[:, moff:SG],
                                         t[:, moff:SG])
                nc.gpsimd.partition_all_reduce(sums[:], acc[:], P, ReduceOp.add)

                def fin(sums=sums, gsl=gsl):
                    inv = ap_.tile([P, SG], f32, tag="inv", bufs=2)
                    nc.vector.reciprocal_approx_fast(inv[:], sums[:])
                    nc.vector.tensor_mul(ctx_sb[:, gsl], ctx_sb[:, gsl], inv[:])
                fin_q.append(fin)
                if drain_ropes and rope_q:
                    flush(rope_q, 2)

        # ---------------- schedule ----------------
        xs0 = stream_x(0)
        sweep(0, xs0)
        xs1 = stream_x(1)
        load_mb()
        sweep(1, xs1)        # + RoPE(0) interleaved
        xs2 = stream_x(2)
        pass_b(0)
        load_wo()
        sweep(2, xs2)        # + RoPE(1), fins(b0)
        xs3 = stream_x(3)
        pass_b(1)
        sweep(3, xs3)        # + RoPE(2), fins(b1)
        pass_b(2, drain_ropes=True)   # RoPE(3) rides b2's DVE slack
        flush(rope_q)
        pass_b(3)

        # ---------------- pass C: out = ctx @ wo (partial) ----------------
        for m in range(NSK):
            if fin_q:
                flush(fin_q, 1)
            for half2 in range(2):
                orow = cp.tile([P, D // 2], bf, tag="orow", bufs=2)
                for n2 in range(0, D // SG // 2, 2):
                    n = half2 * (D // SG // 2) + n2
                    pop = ps.tile([P, 2 * SG], f32, tag="pair", bufs=3,
                                  name="pop")
                    po = [pop[:, 0:SG], pop[:, SG:2 * SG]]
                    for kk in range(NH_LOC):
                        for i in range(2):
                            nc.tensor.matmul(
                                po[i],
                                ctx_sb[:, kk * S + m * P:kk * S + (m + 1) * P],
                                wo_sb[:, ((n + i) * NH_LOC + kk) * SG:
                                         ((n + i) * NH_LOC + kk + 1) * SG],
                                start=(kk == 0), stop=(kk == NH_LOC - 1))
                    for i in range(2):
                        dst = orow[:, (n2 + i) * SG:(n2 + i + 1) * SG]
                        if i:
                            nc.scalar.copy(dst, po[i])
                        else:
                            nc.vector.tensor_copy(dst, po[i])
                nc.sync.dma_start(
                    out_d[m * P:(m + 1) * P,
                          half2 * (D // 2):(half2 + 1) * (D // 2)],
                    orow[:])

    nc.compile()
    return nc


def _host_prep(x, wq, wk, wv, wo, freqs_cos, freqs_sin):
    """Build per-core input maps (all layouts pre-tiled for contiguous DMA)."""
    from concourse import mybir
    BF = np.dtype(mybir.dt.np(mybir.dt.bfloat16))

    x = np.ascontiguousarray(np.asarray(x, dtype=np.float32).reshape(S, D))
    wq = np.asarray(wq, dtype=np.float32)
    wk = np.asarray(wk, dtype=np.float32)
    wv = np.asarray(wv, dtype=np.float32)
    wo = np.asarray(wo, dtype=np.float32)

    perm = np.concatenate([np.arange(0, HD, 2), np.arange(1, HD, 2)])
    scale = 1.0 / math.sqrt(HD)
    wq_p = (wq.reshape(D, N_HEADS, HD)[:, :, perm] * scale).astype(np.float32)
    wk_p = wk.reshape(D, N_KV, HD)[:, :, perm]

    xtb = np.ascontiguousarray(
        x.T.reshape(KT, P, NG, SG).transpose(1, 2, 0, 3)).astype(BF)
    xtb = np.ascontiguousarray(xtb.reshape(P, NG * KT * SG))
    cosT = np.asarray(freqs_cos, np.float32).T
    sinT = np.asarray(freqs_sin, np.float32).T
    cs = np.ascontiguousarray(
        np.concatenate([np.concatenate([cosT, sinT], axis=0),
                        np.concatenate([sinT, cosT], axis=0)],
                       axis=1)).astype(BF)

    in_maps = []
    for c in range(N_CORES):
        wq_c = wq_p[:, 4 * c:4 * c + 4, :].reshape(D, NH_LOC * HD)
        wq_l = np.ascontiguousarray(
            wq_c.reshape(KT, P, NH_LOC * HD).transpose(1, 0, 2)
            .reshape(P, KT * NH_LOC * HD)).astype(BF)
        wk_c = wk_p[:, c, :]
        wk_l = np.ascontiguousarray(
            wk_c.reshape(KT, P, HD).transpose(1, 0, 2).reshape(P, KT * HD)).astype(BF)
        wv_c = wv.reshape(D, N_KV, HD)[:, c, :]
        wv_l = np.ascontiguousarray(
            wv_c.reshape(KT, P, HD).transpose(1, 0, 2).reshape(P, KT * HD)).astype(BF)
        wo_c = wo[4 * c * HD:(4 * c + 4) * HD, :]       # [512, D]
        wo_l = np.ascontiguousarray(
            wo_c.reshape(NH_LOC, P, D // SG, SG).transpose(1, 2, 0, 3)
            .reshape(P, (D // SG) * NH_LOC * SG)).astype(BF)
        in_maps.append({"xt": xtb, "wq": wq_l, "wk": wk_l,
                        "wv": wv_l, "wo": wo_l, "cs": cs})
    return in_maps


def _run(x, wq, wk, wv, wo, freqs_cos, freqs_sin, mask, start_pos, trace=False):
    assert int(start_pos) == 0
    from concourse import mybir
    BF = np.dtype(mybir.dt.np(mybir.dt.bfloat16))
    sk_lists, patterns = _classify_mask(np.asarray(mask, dtype=np.float32))
    n_pat = len(patterns)
    pat_offs = [_diag_off(p) for p in patterns]
    fp = (tuple(tuple(lst) for lst in sk_lists), n_pat, tuple(pat_offs))

    if fp not in _CACHE:
        _CACHE[fp] = _build_program(sk_lists, n_pat, pat_offs)
    nc = _CACHE[fp]

    in_maps = _host_prep(x, wq, wk, wv, wo, freqs_cos, freqs_sin)
    if n_pat:
        mb = np.ascontiguousarray(np.stack(patterns)).astype(BF)
        for m in in_maps:
            m["mb"] = mb

    from concourse.bass_utils import run_bass_kernel_spmd
    res = run_bass_kernel_spmd(nc, in_maps, list(range(N_CORES)), trace=trace)
    out = np.zeros((S, D), dtype=np.float32)
    for c in range(N_CORES):
        out += res.results[c]["out"].astype(np.float32)
    return out.reshape(1, S, D), res


def kernel(x, wq, wk, wv, wo, freqs_cos, freqs_sin, mask, start_pos):
    out, _ = _run(x, wq, wk, wv, wo, freqs_cos, freqs_sin, mask, start_pos)
    return out


# revision 15
# speedup vs baseline: 1.4282x; 1.4282x over previous
"""Trainium2 Bass kernel for Llama-style GQA attention (B=1, S=2048, D=4096,
32 Q heads / 8 KV heads, head_dim 128, RoPE, additive mask, causal-aware).

Sharding: 8-way tensor-parallel over heads. Core c computes Q heads 4c..4c+3
and KV head c end-to-end; the host sums the 8 partial [S, D] outputs (the
all-reduce of the row-parallel wo).

v4 strategy (over v3): the PE runs ONLY real matmuls -- the softmax
denominator and its broadcast are moved off the PE entirely:
  - Fused per-group sweep: K, V and the 4 Q heads accumulate in one k-loop
    (6 matmuls per k-tile, 3 PSUM pairs: kv + 2 q-pairs).  x is streamed
    once per group and each chunk's buffer frees early (smooth prefetch).
  - Post-sweep, ACT stages the 3 PSUM pairs to SBUF bf16 fast (pair slots
    recycle in ~1us); RoPE runs later from the copies (all-bf16, 2x DVE
    rate) interleaved into the NEXT PE-heavy phase where the DVE is idle.
    V dma-transposes ride the idle sync ring.
  - Softmax sum: DVE adds each exp-pair (bf16), GPSIMD accumulates the
    pair-sums (fp32) and partition_all_reduce broadcasts the key-sum to
    all partitions; DVE reciprocal + in-place normalize of the context.
    The context is staged out of PSUM unnormalized (DVE copy) so the PSUM
    bank ring never waits on the slow gpsimd chain; reciprocal+multiply
    are emitted one phase later (deadline: pass C reads ctx much later).
  - Diagonal mask tiles at offsets 256/384 are computed at half width
    (columns below the offset are fully masked); their exp(mask) patterns
    make full-width reads exact where needed.
  - Schedule: sweep(G+1) is emitted before attention(G) so RoPE/copies
    have a whole phase of slack; pass C (wo) gets 3-deep PSUM pipelining.
"""

import math
import os
import numpy as np

os.environ.setdefault("NEURON_RT_RESET_CORES", "1")

P = 128          # SBUF partitions / head_dim / tile edge
S = 2048         # sequence length
D = 4096         # model dim
HD = 128         # head dim
N_HEADS = 32
N_KV = 8
N_CORES = 8
NH_LOC = N_HEADS // N_CORES   # 4 local Q heads
SG = 512         # score/free-dim group width (one PSUM bank of fp32)
NG = S // SG     # 4 q-position groups
KT = D // P      # 32 contraction tiles for projections
NSK = S // P     # 16 key tiles

_CACHE = {}


def _classify_mask(mask):
    """Classify each [P, SG] block of mask.T into skip / plain / masked."""
    mt = np.ascontiguousarray(mask.T.astype(np.float32))
    patterns = []
    pat_idx = {}
    sk_lists = []
    for G in range(NG):
        lst = []
        for m in range(NSK):
            blk = mt[m * P:(m + 1) * P, G * SG:(G + 1) * SG]
            if np.all(np.isneginf(blk)):
                continue
            if np.all(blk == 0.0):
                lst.append((m, None))
                continue
            with np.errstate(over="ignore"):
                pat = np.exp(blk).astype(np.float32)
            key = pat.tobytes()
            if key not in pat_idx:
                pat_idx[key] = len(patterns)
                patterns.append(pat)
            lst.append((m, pat_idx[key]))
        sk_lists.append(lst)
    return sk_lists, patterns


def _diag_off(pat_np, thresh=256):
    """Column offset below which a pattern block is entirely zero (0 if the
    leading-zero span is < thresh; offsets are quantized to {0, 256})."""
    colmax = pat_np.max(axis=0)
    nz = np.nonzero(colmax)[0]
    first = int(nz[0]) if len(nz) else pat_np.shape[1]
    return 256 if first >= thresh else 0


def _build_program(sk_lists, n_pat, pat_offs):
    import concourse.tile as tile
    from concourse import bacc, mybir
    from concourse.bass_isa import ReduceOp
    from contextlib import ExitStack

    f32 = mybir.dt.float32
    bf = mybir.dt.bfloat16
    Exp = mybir.ActivationFunctionType.Exp

    nc = bacc.Bacc()
    xt_d = nc.dram_tensor("xt", [P, NG * KT * SG], bf, kind="ExternalInput")
    wq_d = nc.dram_tensor("wq", [P, KT * NH_LOC * HD], bf, kind="ExternalInput")
    wk_d = nc.dram_tensor("wk", [P, KT * HD], bf, kind="ExternalInput")
    wv_d = nc.dram_tensor("wv", [P, KT * HD], bf, kind="ExternalInput")
    wo_d = nc.dram_tensor("wo", [P, (D // SG) * NH_LOC * SG], bf,
                          kind="ExternalInput")
    # [cos;sin] in cols [0:S], [sin;cos] in cols [S:2S] -- both partition
    # layouts, because DVE tensor_tensor requires equal base partitions
    # when both inputs are in SBUF
    cs_d = nc.dram_tensor("cs", [P, 2 * S], bf, kind="ExternalInput")
    mb_d = None
    if n_pat:
        mb_d = nc.dram_tensor("mb", [n_pat, P, SG], bf, kind="ExternalInput")
    out_d = nc.dram_tensor("out", [S, D], bf, kind="ExternalOutput")

    XCH = 4 * SG     # xT DMA chunk: 4 k-tiles, 4KB per partition line

    with ExitStack() as ctx:
        tc = ctx.enter_context(tile.TileContext(nc))
        consts = ctx.enter_context(tc.tile_pool(name="consts", bufs=1))
        kv = ctx.enter_context(tc.tile_pool(name="kv", bufs=1))
        xp = ctx.enter_context(tc.tile_pool(name="xp", bufs=8))
        qp = ctx.enter_context(tc.tile_pool(name="qp", bufs=8))
        rp = ctx.enter_context(tc.tile_pool(name="rp", bufs=4))
        ep = ctx.enter_context(tc.tile_pool(name="ep", bufs=4))
        sp = ctx.enter_context(tc.tile_pool(name="sp", bufs=4))
        ap_ = ctx.enter_context(tc.tile_pool(name="ap", bufs=8))
        cp = ctx.enter_context(tc.tile_pool(name="cp", bufs=4))
        ps = ctx.enter_context(tc.tile_pool(name="ps", bufs=8, space="PSUM"))

        # resident weights / constants on the scalar ring; the xT stream and
        # V transposes own the sync ring.
        wk_sb = consts.tile([P, KT * HD], bf)
        wv_sb = consts.tile([P, KT * HD], bf)
        wq_sb = consts.tile([P, KT * NH_LOC * HD], bf)
        # leads so the first k-tiles' matmuls start as early as possible
        nc.scalar.dma_start(wk_sb[:, 0:2 * HD], wk_d[:, 0:2 * HD])
        nc.scalar.dma_start(wv_sb[:, 0:2 * HD], wv_d[:, 0:2 * HD])
        wqt = NH_LOC * HD  # per-k-tile wq block
        nc.scalar.dma_start(wq_sb[:, 0:2 * wqt], wq_d[:, 0:2 * wqt])
        half = KT * HD // 2
        nc.scalar.dma_start(wk_sb[:, 2 * HD:half], wk_d[:, 2 * HD:half])
        nc.scalar.dma_start(wv_sb[:, 2 * HD:half], wv_d[:, 2 * HD:half])
        nc.scalar.dma_start(wk_sb[:, half:], wk_d[:, half:])
        nc.scalar.dma_start(wv_sb[:, half:], wv_d[:, half:])
        qqt = KT * NH_LOC * HD // 8
        for i in range(8):
            lo = max(i * qqt, 2 * wqt)
            if lo < (i + 1) * qqt:
                nc.scalar.dma_start(wq_sb[:, lo:(i + 1) * qqt],
                                    wq_d[:, lo:(i + 1) * qqt])
        cs_sb = consts.tile([P, 2 * S], bf)
        nc.scalar.dma_start(cs_sb[:, 0:S], cs_d[:, 0:S])
        nc.scalar.dma_start(cs_sb[:, S:2 * S], cs_d[:, S:2 * S])
        mb_sb = None
        if n_pat:
            mb_sb = consts.tile([P, n_pat * SG], bf, name="mb_sb")
        wo_sb = consts.tile([P, (D // SG) * NH_LOC * SG], bf)

        def load_mb():
            for i in range(n_pat):
                nc.scalar.dma_start(mb_sb[:, i * SG:(i + 1) * SG], mb_d[i])

        def load_wo():
            for i in range(8):
                nc.scalar.dma_start(wo_sb[:, i * qqt:(i + 1) * qqt],
                                    wo_d[:, i * qqt:(i + 1) * qqt])

        # full-sequence KV + context accumulators
        kT_sb = kv.tile([P, S], bf)                  # [head_dim', s]
        v_sb = kv.tile([P, S], bf)                   # [s%P, (s//P)*HD + hd]
        ctx_sb = kv.tile([P, NH_LOC * S], bf)        # [hd, h*S + sq]

        def stream_x(G):
            """DMA the 8 xT chunks of group G; returns per-k slices."""
            slices = []
            for c2 in range(KT * SG // XCH):
                xw = xp.tile([P, XCH], bf, tag="xt", bufs=8, name="xt")
                blk = G * KT * SG + c2 * XCH
                if G == 0 and c2 == 0:
                    nc.sync.dma_start(xw[:, 0:SG], xt_d[:, blk:blk + SG])
                    nc.sync.dma_start(xw[:, SG:XCH],
                                      xt_d[:, blk + SG:blk + XCH])
                else:
                    nc.sync.dma_start(xw[:], xt_d[:, blk:blk + XCH])
                for j in range(XCH // SG):
                    slices.append(xw[:, j * SG:(j + 1) * SG])
            return slices

        def rope(src, dr, di, G):
            cos0 = cs_sb[0:64, G * SG:(G + 1) * SG]
            sin64 = cs_sb[64:128, G * SG:(G + 1) * SG]
            sin0 = cs_sb[0:64, S + G * SG:S + (G + 1) * SG]
            cos64 = cs_sb[64:128, S + G * SG:S + (G + 1) * SG]
            ta = rp.tile([64, SG], bf, tag="ropeA", bufs=2)
            tb = rp.tile([64, SG], bf, tag="ropeB", bufs=2)
            nc.vector.tensor_mul(ta[:], src[0:64, :], cos0)
            nc.vector.tensor_mul(tb[:], src[64:128, :], sin64)
            nc.vector.tensor_sub(dr, ta[:], tb[:])
            tc2 = rp.tile([64, SG], bf, tag="ropeA", bufs=2)
            td = rp.tile([64, SG], bf, tag="ropeB", bufs=2)
            nc.vector.tensor_mul(tc2[:], src[0:64, :], sin0)
            nc.vector.tensor_mul(td[:], src[64:128, :], cos64)
            nc.vector.tensor_add(di, tc2[:], td[:])

        qts = {}
        rope_q = []      # DVE: RoPE of the latest sweep (run in next phase)
        fin_q = []       # DVE: reciprocal + in-place ctx normalize

        def flush(queue, n=None):
            todo = queue[:n] if n is not None else queue[:]
            del queue[:len(todo)]
            for t in todo:
                t()

        def sweep(G, xs):
            """Fused K/V/Q projections for s-group G: 6 matmuls per k-tile."""
            pkv = ps.tile([P, 2 * SG], f32, tag="pair", bufs=3, name="pkv")
            pq1 = ps.tile([P, 2 * SG], f32, tag="pair", bufs=3, name="pq1")
            pq2 = ps.tile([P, 2 * SG], f32, tag="pair", bufs=3, name="pq2")
            dsts = [pkv[:, 0:SG], pkv[:, SG:2 * SG],
                    pq1[:, 0:SG], pq1[:, SG:2 * SG],
                    pq2[:, 0:SG], pq2[:, SG:2 * SG]]
            for k in range(KT):
                st, sp_ = (k == 0), (k == KT - 1)
                nc.tensor.matmul(dsts[0], wk_sb[:, k * HD:(k + 1) * HD],
                                 xs[k], start=st, stop=sp_)
                nc.tensor.matmul(dsts[1], wv_sb[:, k * HD:(k + 1) * HD],
                                 xs[k], start=st, stop=sp_)
                for l in range(NH_LOC):
                    nc.tensor.matmul(
                        dsts[2 + l],
                        wq_sb[:, (k * NH_LOC + l) * HD:(k * NH_LOC + l + 1) * HD],
                        xs[k], start=st, stop=sp_)
                # previous sweep's deferred DVE work rides this PE-heavy loop
                if k % 6 == 5 and rope_q:
                    flush(rope_q, 1)
                if k % 8 == 7 and fin_q:
                    flush(fin_q, 1)
            # stage PSUM -> SBUF bf16 promptly on ACT: pair slots recycle in
            # ~1us each so the next phase's stp allocations never stall long
            kvc = sp.tile([P, 2 * SG], bf, tag="swcp", bufs=3, name="kvc")
            q1c = sp.tile([P, 2 * SG], bf, tag="swcp", bufs=3, name="q1c")
            q2c = sp.tile([P, 2 * SG], bf, tag="swcp", bufs=3, name="q2c")
            nc.scalar.copy(kvc[:], pkv[:])
            nc.scalar.copy(q1c[:], pq1[:])
            nc.scalar.copy(q2c[:], pq2[:])
            # V transposes on the idle sync ring (deadline: attention(G))
            for j in range(SG // P):
                nc.sync.dma_start_transpose(
                    v_sb[:, (G * 4 + j) * HD:(G * 4 + j + 1) * HD],
                    kvc[:, SG + j * P:SG + (j + 1) * P])
            # RoPE from the SBUF copies, deferred into the next PE phase
            def rope_k(G=G, kvc=kvc):
                gsl = slice(G * SG, (G + 1) * SG)
                rope(kvc[:, 0:SG], kT_sb[0:64, gsl], kT_sb[64:128, gsl], G)
            rope_q.append(rope_k)
            for i, (src, lo) in enumerate(((q1c, 0), (q1c, SG),
                                           (q2c, 0), (q2c, SG))):
                dst = qp.tile([P, SG], bf, tag="qT", bufs=8, name="qT")
                qts[(G, i)] = dst
                def rope_qh(src=src, lo=lo, dst=dst, G=G):
                    rope(src[:, lo:lo + SG], dst[0:64, :], dst[64:128, :], G)
                rope_q.append(rope_qh)

        def pass_b(G, drain_ropes=False):
            # order: diagonal tiles first (d0 full-width carries start=True),
            # then full tiles; the LAST tile is full-width (carries stop) --
            # for G0 that's the offset-384 diag with its dead region zeroed.
            diag = [e for e in sk_lists[G] if e[1] is not None]
            plain = [e for e in sk_lists[G] if e[1] is None]
            diag.sort(key=lambda e: e[0])
            # (m, pat, col_off)
            tiles = [(m, pat, pat_offs[pat]) for m, pat in diag] + \
                    [(m, pat, 0) for m, pat in plain]
            n_sk = len(tiles)
            npair = (n_sk + 1) // 2
            g0 = not plain
            for h in range(NH_LOC):
                cacc = ps.tile([P, SG], f32, tag="bank", bufs=2, name="cacc")
                # bufs=4: the reciprocal reading sums(h) is emitted a phase
                # later, so all 4 heads' sums must stay live
                sums = ap_.tile([P, SG], f32, tag="sums", bufs=4)
                # eager balanced fold of pair-sums on the DVE: gpsimd gets
                # exactly ONE all-reduce per head (its tensor ops are ~4x
                # slower than DVE and a per-pair gpsimd chain stalls the
                # whole pipeline through the tp ring + strict FIFOs)
                tstack = []   # [tile, level]
                part_ts = []

                def fold_push(t):
                    tstack.append([t, 0])
                    while (len(tstack) > 2
                           and tstack[-1][1] == tstack[-2][1]):
                        b = tstack.pop()
                        a = tstack.pop()
                        nt = ap_.tile([P, SG], bf, tag="tp", bufs=8)
                        nc.vector.tensor_add(nt[:], a[0][:], b[0][:])
                        tstack.append([nt, max(a[1], b[1]) + 1])

                def emit_pair(p):
                    w = 2 if 2 * p + 1 < n_sk else 1
                    pr = tiles[2 * p:2 * p + w]
                    stp = ps.tile([P, 2 * SG], f32, tag="pair", bufs=3,
                                  name="stp")
                    for j, (m, pat, off) in enumerate(pr):
                        nc.tensor.matmul(stp[:, j * SG + off:(j + 1) * SG],
                                         kT_sb[:, m * P:(m + 1) * P],
                                         qts[(G, h)][:, off:SG],
                                         start=True, stop=True)
                    ex = ep.tile([P, 2 * SG], bf, tag="ex", bufs=4)
                    offs = [t[2] for t in pr]
                    if w == 2 and offs[0] == 0 and offs[1] == 0:
                        nc.scalar.activation(ex[:, 0:2 * SG], stp[:, 0:2 * SG],
                                             Exp)
                    else:
                        for j, (m, pat, off) in enumerate(pr):
                            nc.scalar.activation(
                                ex[:, j * SG + off:(j + 1) * SG],
                                stp[:, j * SG + off:(j + 1) * SG], Exp)
                    if g0 and p == npair - 1:
                        # last G0 tile runs its PV full-width to carry the
                        # stop flag; zero its dead region
                        nc.vector.memset(ex[:, SG:SG + offs[1]], 0.0)
                    for j, (m, pat, off) in enumerate(pr):
                        if pat is not None:
                            nc.vector.tensor_mul(
                                ex[:, j * SG + off:(j + 1) * SG],
                                ex[:, j * SG + off:(j + 1) * SG],
                                mb_sb[:, pat * SG + off:(pat + 1) * SG])
                    # softmax-denominator pair-sum on the DVE (bf16); fp32
                    # accumulation chain runs on gpsimd -- no PE involvement
                    t = ap_.tile([P, SG], bf, tag="tp", bufs=8)
                    moff = max(offs)
                    if w == 2:
                        nc.vector.tensor_add(t[:, moff:SG],
                                             ex[:, moff:SG],
                                             ex[:, SG + moff:2 * SG])
                    else:
                        nc.vector.tensor_copy(t[:, moff:SG], ex[:, moff:SG])
                    if moff:
                        part_ts.append((t, moff))
                    else:
                        fold_push(t)
                    return ex

                exq = [emit_pair(p) for p in range(min(3, npair))]
                for p in range(npair):
                    ex = exq[p]
                    w = 2 if 2 * p + 1 < n_sk else 1
                    for j, (m, pat, off) in enumerate(tiles[2 * p:2 * p + w]):
                        idx = 2 * p + j
                        if g0 and idx == n_sk - 1:
                            off = 0      # full-width PV carries the stop
                        nc.tensor.matmul(cacc[:, off:SG],
                                         v_sb[:, m * HD:(m + 1) * HD],
                                         ex[:, j * SG + off:(j + 1) * SG],
                                         start=(idx == 0), stop=(idx == n_sk - 1))
                    if p + 3 < npair:
                        exq.append(emit_pair(p + 3))
                # stage the unnormalized context out of PSUM so the bank
                # ring never waits on the softmax-sum chain
                gsl = slice(h * S + G * SG, h * S + (G + 1) * SG)
                nc.vector.tensor_copy(ctx_sb[:, gsl], cacc[:])
                # collapse the fold stack into one [P, SG] tile on the DVE
                while len(tstack) > 1:
                    b = tstack.pop()
                    a = tstack.pop()
                    dst = ap_.tile([P, SG], bf,
                                   tag="th" if len(tstack) == 0 else "tp",
                                   bufs=4 if len(tstack) == 0 else 8)
                    nc.vector.tensor_add(dst[:], a[0][:], b[0][:])
                    tstack.append([dst, max(a[1], b[1]) + 1])
                th = tstack.pop()[0]
                for t, moff in part_ts:
                    nc.vector.tensor_add(th[:, moff:SG], th[:, moff:SG],
                                         t[:, moff:SG])
                nc.gpsimd.partition_all_reduce(sums[:], th[:], P, ReduceOp.add)

                def fin(sums=sums, gsl=gsl):
                    inv = ap_.tile([P, SG], f32, tag="inv", bufs=2)
                    nc.vector.reciprocal_approx_fast(inv[:], sums[:])
                    nc.vector.tensor_mul(ctx_sb[:, gsl], ctx_sb[:, gsl], inv[:])
                fin_q.append(fin)
                if drain_ropes and rope_q:
                    flush(rope_q, 2)

        # ---------------- schedule ----------------
        xs0 = stream_x(0)
        sweep(0, xs0)
        xs1 = stream_x(1)
        load_mb()
        sweep(1, xs1)        # + RoPE(0) interleaved
        xs2 = stream_x(2)
        pass_b(0)
        load_wo()
        sweep(2, xs2)        # + RoPE(1), fins(b0)
        xs3 = stream_x(3)
        pass_b(1)
        sweep(3, xs3)        # + RoPE(2), fins(b1)
        pass_b(2, drain_ropes=True)   # RoPE(3) rides b2's DVE slack
        flush(rope_q)
        pass_b(3)

        # ---------------- pass C: out = ctx @ wo (partial) ----------------
        for m in range(NSK):
            if fin_q:
                flush(fin_q, 1)
            for half2 in range(2):
                orow = cp.tile([P, D // 2], bf, tag="orow", bufs=2)
                for n2 in range(0, D // SG // 2, 2):
                    n = half2 * (D // SG // 2) + n2
                    pop = ps.tile([P, 2 * SG], f32, tag="pair", bufs=3,
                                  name="pop")
                    po = [pop[:, 0:SG], pop[:, SG:2 * SG]]
                    for kk in range(NH_LOC):
                        for i in range(2):
                            nc.tensor.matmul(
                                po[i],
                                ctx_sb[:, kk * S + m * P:kk * S + (m + 1) * P],
                                wo_sb[:, ((n + i) * NH_LOC + kk) * SG:
                                         ((n + i) * NH_LOC + kk + 1) * SG],
                                start=(kk == 0), stop=(kk == NH_LOC - 1))
                    for i in range(2):
                        dst = orow[:, (n2 + i) * SG:(n2 + i + 1) * SG]
                        if i:
                            nc.scalar.copy(dst, po[i])
                        else:
                            nc.vector.tensor_copy(dst, po[i])
                nc.sync.dma_start(
                    out_d[m * P:(m + 1) * P,
                          half2 * (D // 2):(half2 + 1) * (D // 2)],
                    orow[:])

    nc.compile()
    return nc


def _host_prep(x, wq, wk, wv, wo, freqs_cos, freqs_sin):
    """Build per-core input maps (all layouts pre-tiled for contiguous DMA)."""
    from concourse import mybir
    BF = np.dtype(mybir.dt.np(mybir.dt.bfloat16))

    x = np.ascontiguousarray(np.asarray(x, dtype=np.float32).reshape(S, D))
    wq = np.asarray(wq, dtype=np.float32)
    wk = np.asarray(wk, dtype=np.float32)
    wv = np.asarray(wv, dtype=np.float32)
    wo = np.asarray(wo, dtype=np.float32)

    perm = np.concatenate([np.arange(0, HD, 2), np.arange(1, HD, 2)])
    scale = 1.0 / math.sqrt(HD)
    wq_p = (wq.reshape(D, N_HEADS, HD)[:, :, perm] * scale).astype(np.float32)
    wk_p = wk.reshape(D, N_KV, HD)[:, :, perm]

    xtb = np.ascontiguousarray(
        x.T.reshape(KT, P, NG, SG).transpose(1, 2, 0, 3)).astype(BF)
    xtb = np.ascontiguousarray(xtb.reshape(P, NG * KT * SG))
    cosT = np.asarray(freqs_cos, np.float32).T
    sinT = np.asarray(freqs_sin, np.float32).T
    cs = np.ascontiguousarray(
        np.concatenate([np.concatenate([cosT, sinT], axis=0),
                        np.concatenate([sinT, cosT], axis=0)],
                       axis=1)).astype(BF)

    in_maps = []
    for c in range(N_CORES):
        wq_c = wq_p[:, 4 * c:4 * c + 4, :].reshape(D, NH_LOC * HD)
        wq_l = np.ascontiguousarray(
            wq_c.reshape(KT, P, NH_LOC * HD).transpose(1, 0, 2)
            .reshape(P, KT * NH_LOC * HD)).astype(BF)
        wk_c = wk_p[:, c, :]
        wk_l = np.ascontiguousarray(
            wk_c.reshape(KT, P, HD).transpose(1, 0, 2).reshape(P, KT * HD)).astype(BF)
        wv_c = wv.reshape(D, N_KV, HD)[:, c, :]
        wv_l = np.ascontiguousarray(
            wv_c.reshape(KT, P, HD).transpose(1, 0, 2).reshape(P, KT * HD)).astype(BF)
        wo_c = wo[4 * c * HD:(4 * c + 4) * HD, :]       # [512, D]
        wo_l = np.ascontiguousarray(
            wo_c.reshape(NH_LOC, P, D // SG, SG).transpose(1, 2, 0, 3)
            .reshape(P, (D // SG) * NH_LOC * SG)).astype(BF)
        in_maps.append({"xt": xtb, "wq": wq_l, "wk": wk_l,
                        "wv": wv_l, "wo": wo_l, "cs": cs})
    return in_maps


def _run(x, wq, wk, wv, wo, freqs_cos, freqs_sin, mask, start_pos, trace=False):
    assert int(start_pos) == 0
    from concourse import mybir
    BF = np.dtype(mybir.dt.np(mybir.dt.bfloat16))
    sk_lists, patterns = _classify_mask(np.asarray(mask, dtype=np.float32))
    n_pat = len(patterns)
    pat_offs = [_diag_off(p) for p in patterns]
    fp = (tuple(tuple(lst) for lst in sk_lists), n_pat, tuple(pat_offs))

    if fp not in _CACHE:
        _CACHE[fp] = _build_program(sk_lists, n_pat, pat_offs)
    nc = _CACHE[fp]

    in_maps = _host_prep(x, wq, wk, wv, wo, freqs_cos, freqs_sin)
    if n_pat:
        mb = np.ascontiguousarray(np.stack(patterns)).astype(BF)
        for m in in_maps:
            m["mb"] = mb

    from concourse.bass_utils import run_bass_kernel_spmd
    res = run_bass_kernel_spmd(nc, in_maps, list(range(N_CORES)), trace=trace)
    out = np.zeros((S, D), dtype=np.float32)
    for c in range(N_CORES):
        out += res.results[c]["out"].astype(np.float32)
    return out.reshape(1, S, D), res


def kernel(x, wq, wk, wv, wo, freqs_cos, freqs_sin, mask, start_pos):
    out, _ = _run(x, wq, wk, wv, wo, freqs_cos, freqs_sin, mask, start_pos)
    return out


# revision 22
# speedup vs baseline: 1.4455x; 1.0121x over previous
"""Trainium2 Bass kernel for Llama-style GQA attention (B=1, S=2048, D=4096,
32 Q heads / 8 KV heads, head_dim 128, RoPE, additive mask, causal-aware).

Sharding: 8-way tensor-parallel over heads. Core c computes Q heads 4c..4c+3
and KV head c end-to-end; the host sums the 8 partial [S, D] outputs (the
all-reduce of the row-parallel wo).

v4 strategy (over v3): the PE runs ONLY real matmuls -- the softmax
denominator and its broadcast are moved off the PE entirely:
  - Fused per-group sweep: K, V and the 4 Q heads accumulate in one k-loop
    (6 matmuls per k-tile, 3 PSUM pairs: kv + 2 q-pairs).  x is streamed
    once per group and each chunk's buffer frees early (smooth prefetch).
  - Post-sweep, ACT stages the 3 PSUM pairs to SBUF bf16 fast (pair slots
    recycle in ~1us); RoPE runs later from the copies (all-bf16, 2x DVE
    rate) interleaved into the NEXT PE-heavy phase where the DVE is idle.
    V dma-transposes ride the idle sync ring.
  - Softmax sum: DVE adds each exp-pair (bf16), GPSIMD accumulates the
    pair-sums (fp32) and partition_all_reduce broadcasts the key-sum to
    all partitions; DVE reciprocal + in-place normalize of the context.
    The context is staged out of PSUM unnormalized (DVE copy) so the PSUM
    bank ring never waits on the slow gpsimd chain; reciprocal+multiply
    are emitted one phase later (deadline: pass C reads ctx much later).
  - Diagonal mask tiles at offsets 256/384 are computed at half width
    (columns below the offset are fully masked); their exp(mask) patterns
    make full-width reads exact where needed.
  - Schedule: sweep(G+1) is emitted before attention(G) so RoPE/copies
    have a whole phase of slack; pass C (wo) gets 3-deep PSUM pipelining.
"""

import math
import os
import numpy as np

os.environ.setdefault("NEURON_RT_RESET_CORES", "1")

P = 128          # SBUF partitions / head_dim / tile edge
S = 2048         # sequence length
D = 4096         # model dim
HD = 128         # head dim
N_HEADS = 32
N_KV = 8
N_CORES = 8
NH_LOC = N_HEADS // N_CORES   # 4 local Q heads
SG = 512         # score/free-dim group width (one PSUM bank of fp32)
NG = S // SG     # 4 q-position groups
KT = D // P      # 32 contraction tiles for projections
NSK = S // P     # 16 key tiles

_CACHE = {}


def _classify_mask(mask):
    """Classify each [P, SG] block of mask.T into skip / plain / masked."""
    mt = np.ascontiguousarray(mask.T.astype(np.float32))
    patterns = []
    pat_idx = {}
    sk_lists = []
    for G in range(NG):
        lst = []
        for m in range(NSK):
            blk = mt[m * P:(m + 1) * P, G * SG:(G + 1) * SG]
            if np.all(np.isneginf(blk)):
                continue
            if np.all(blk == 0.0):
                lst.append((m, None))
                continue
            with np.errstate(over="ignore"):
                pat = np.exp(blk).astype(np.float32)
            key = pat.tobytes()
            if key not in pat_idx:
                pat_idx[key] = len(patterns)
                patterns.append(pat)
            lst.append((m, pat_idx[key]))
        sk_lists.append(lst)
    return sk_lists, patterns


def _diag_off(pat_np, thresh=256):
    """Column offset below which a pattern block is entirely zero (0 if the
    leading-zero span is < thresh; offsets are quantized to {0, 256})."""
    colmax = pat_np.max(axis=0)
    nz = np.nonzero(colmax)[0]
    first = int(nz[0]) if len(nz) else pat_np.shape[1]
    return 256 if first >= thresh else 0


def _build_program(sk_lists, n_pat, pat_offs):
    import concourse.tile as tile
    from concourse import bacc, mybir
    from concourse.bass_isa import ReduceOp
    from contextlib import ExitStack

    f32 = mybir.dt.float32
    bf = mybir.dt.bfloat16
    Exp = mybir.ActivationFunctionType.Exp

    nc = bacc.Bacc()
    xt_d = nc.dram_tensor("xt", [P, NG * KT * SG], bf, kind="ExternalInput")
    wq_d = nc.dram_tensor("wq", [P, KT * NH_LOC * HD], bf, kind="ExternalInput")
    wk_d = nc.dram_tensor("wk", [P, KT * HD], bf, kind="ExternalInput")
    wv_d = nc.dram_tensor("wv", [P, KT * HD], bf, kind="ExternalInput")
    wo_d = nc.dram_tensor("wo", [P, (D // SG) * NH_LOC * SG], bf,
                          kind="ExternalInput")
    # [cos;sin] in cols [0:S], [sin;cos] in cols [S:2S] -- both partition
    # layouts, because DVE tensor_tensor requires equal base partitions
    # when both inputs are in SBUF
    cs_d = nc.dram_tensor("cs", [P, 2 * S], bf, kind="ExternalInput")
    mb_d = None
    if n_pat:
        mb_d = nc.dram_tensor("mb", [n_pat, P, SG], bf, kind="ExternalInput")
    out_d = nc.dram_tensor("out", [S, D], bf, kind="ExternalOutput")

    XCH = 4 * SG     # xT DMA chunk: 4 k-tiles, 4KB per partition line

    with ExitStack() as ctx:
        tc = ctx.enter_context(tile.TileContext(nc))
        consts = ctx.enter_context(tc.tile_pool(name="consts", bufs=1))
        kv = ctx.enter_context(tc.tile_pool(name="kv", bufs=1))
        xp = ctx.enter_context(tc.tile_pool(name="xp", bufs=7))
        qp = ctx.enter_context(tc.tile_pool(name="qp", bufs=8))
        rp = ctx.enter_context(tc.tile_pool(name="rp", bufs=4))
        ep = ctx.enter_context(tc.tile_pool(name="ep", bufs=5))
        sp = ctx.enter_context(tc.tile_pool(name="sp", bufs=4))
        ap_ = ctx.enter_context(tc.tile_pool(name="ap", bufs=8))
        cp = ctx.enter_context(tc.tile_pool(name="cp", bufs=4))
        ps = ctx.enter_context(tc.tile_pool(name="ps", bufs=8, space="PSUM"))

        # resident weights / constants on the scalar ring; the xT stream and
        # V transposes own the sync ring.
        wk_sb = consts.tile([P, KT * HD], bf)
        wv_sb = consts.tile([P, KT * HD], bf)
        wq_sb = consts.tile([P, KT * NH_LOC * HD], bf)
        # leads so the first k-tiles' matmuls start as early as possible,
        # then supply in k-range order (the fused sweep consumes wk/wv/wq
        # together, so the stream must interleave them, not batch by tensor)
        nc.scalar.dma_start(wk_sb[:, 0:2 * HD], wk_d[:, 0:2 * HD])
        nc.scalar.dma_start(wv_sb[:, 0:2 * HD], wv_d[:, 0:2 * HD])
        wqt = NH_LOC * HD  # per-k-tile wq block
        nc.scalar.dma_start(wq_sb[:, 0:2 * wqt], wq_d[:, 0:2 * wqt])
        qqt = KT * NH_LOC * HD // 8
        for r in range(4):           # k in [8r, 8r+8)
            klo, khi = 8 * r * HD, 8 * (r + 1) * HD
            klo = max(klo, 2 * HD)
            nc.scalar.dma_start(wk_sb[:, klo:khi], wk_d[:, klo:khi])
            nc.scalar.dma_start(wv_sb[:, klo:khi], wv_d[:, klo:khi])
            for i in (2 * r, 2 * r + 1):
                lo = max(i * qqt, 2 * wqt)
                if lo < (i + 1) * qqt:
                    nc.scalar.dma_start(wq_sb[:, lo:(i + 1) * qqt],
                                        wq_d[:, lo:(i + 1) * qqt])
        cs_sb = consts.tile([P, 2 * S], bf)
        nc.scalar.dma_start(cs_sb[:, 0:S], cs_d[:, 0:S])
        nc.scalar.dma_start(cs_sb[:, S:2 * S], cs_d[:, S:2 * S])
        mb_sb = None
        if n_pat:
            mb_sb = consts.tile([P, n_pat * SG], bf, name="mb_sb")
        wo_sb = consts.tile([P, (D // SG) * NH_LOC * SG], bf)

        def load_mb():
            for i in range(n_pat):
                nc.scalar.dma_start(mb_sb[:, i * SG:(i + 1) * SG], mb_d[i])

        def load_wo():
            for i in range(8):
                nc.scalar.dma_start(wo_sb[:, i * qqt:(i + 1) * qqt],
                                    wo_d[:, i * qqt:(i + 1) * qqt])

        # full-sequence KV + context accumulators
        kT_sb = kv.tile([P, S], bf)                  # [head_dim', s]
        v_sb = kv.tile([P, S], bf)                   # [s%P, (s//P)*HD + hd]
        ctx_sb = kv.tile([P, NH_LOC * S], bf)        # [hd, h*S + sq]

        def stream_x(G):
            """DMA the 8 xT chunks of group G; returns per-k slices."""
            slices = []
            for c2 in range(KT * SG // XCH):
                xw = xp.tile([P, XCH], bf, tag="xt", bufs=7, name="xt")
                blk = G * KT * SG + c2 * XCH
                if G == 0 and c2 == 0:
                    nc.sync.dma_start(xw[:, 0:SG], xt_d[:, blk:blk + SG])
                    nc.sync.dma_start(xw[:, SG:XCH],
                                      xt_d[:, blk + SG:blk + XCH])
                else:
                    nc.sync.dma_start(xw[:], xt_d[:, blk:blk + XCH])
                for j in range(XCH // SG):
                    slices.append(xw[:, j * SG:(j + 1) * SG])
            return slices

        def rope(src, dr, di, G):
            cos0 = cs_sb[0:64, G * SG:(G + 1) * SG]
            sin64 = cs_sb[64:128, G * SG:(G + 1) * SG]
            sin0 = cs_sb[0:64, S + G * SG:S + (G + 1) * SG]
            cos64 = cs_sb[64:128, S + G * SG:S + (G + 1) * SG]
            ta = rp.tile([64, SG], bf, tag="ropeA", bufs=2)
            tb = rp.tile([64, SG], bf, tag="ropeB", bufs=2)
            nc.vector.tensor_mul(ta[:], src[0:64, :], cos0)
            nc.vector.tensor_mul(tb[:], src[64:128, :], sin64)
            nc.vector.tensor_sub(dr, ta[:], tb[:])
            tc2 = rp.tile([64, SG], bf, tag="ropeA", bufs=2)
            td = rp.tile([64, SG], bf, tag="ropeB", bufs=2)
            nc.vector.tensor_mul(tc2[:], src[0:64, :], sin0)
            nc.vector.tensor_mul(td[:], src[64:128, :], cos64)
            nc.vector.tensor_add(di, tc2[:], td[:])

        qts = {}
        rope_q = []      # DVE: RoPE of the latest sweep (run in next phase)
        fin_q = []       # DVE: reciprocal + in-place ctx normalize

        def flush(queue, n=None):
            todo = queue[:n] if n is not None else queue[:]
            del queue[:len(todo)]
            for t in todo:
                t()

        def sweep(G, xs):
            """Fused K/V/Q projections for s-group G: 6 matmuls per k-tile."""
            pkv = ps.tile([P, 2 * SG], f32, tag="pair", bufs=3, name="pkv")
            pq1 = ps.tile([P, 2 * SG], f32, tag="pair", bufs=3, name="pq1")
            pq2 = ps.tile([P, 2 * SG], f32, tag="pair", bufs=3, name="pq2")
            dsts = [pkv[:, 0:SG], pkv[:, SG:2 * SG],
                    pq1[:, 0:SG], pq1[:, SG:2 * SG],
                    pq2[:, 0:SG], pq2[:, SG:2 * SG]]
            for k in range(KT):
                st, sp_ = (k == 0), (k == KT - 1)
                nc.tensor.matmul(dsts[0], wk_sb[:, k * HD:(k + 1) * HD],
                                 xs[k], start=st, stop=sp_)
                nc.tensor.matmul(dsts[1], wv_sb[:, k * HD:(k + 1) * HD],
                                 xs[k], start=st, stop=sp_)
                for l in range(NH_LOC):
                    nc.tensor.matmul(
                        dsts[2 + l],
                        wq_sb[:, (k * NH_LOC + l) * HD:(k * NH_LOC + l + 1) * HD],
                        xs[k], start=st, stop=sp_)
                # previous sweep's deferred DVE work rides this PE-heavy loop
                if k % 6 == 5 and rope_q:
                    flush(rope_q, 1)
                if k % 8 == 7 and fin_q:
                    flush(fin_q, 1)
            # stage PSUM -> SBUF bf16 promptly on ACT: pair slots recycle in
            # ~1us each so the next phase's stp allocations never stall long
            kvc = sp.tile([P, 2 * SG], bf, tag="swcp", bufs=3, name="kvc")
            q1c = sp.tile([P, 2 * SG], bf, tag="swcp", bufs=3, name="q1c")
            q2c = sp.tile([P, 2 * SG], bf, tag="swcp", bufs=3, name="q2c")
            nc.scalar.copy(kvc[:], pkv[:])
            nc.scalar.copy(q1c[:], pq1[:])
            nc.scalar.copy(q2c[:], pq2[:])
            # V transposes on the idle sync ring (deadline: attention(G))
            for j in range(SG // P):
                nc.sync.dma_start_transpose(
                    v_sb[:, (G * 4 + j) * HD:(G * 4 + j + 1) * HD],
                    kvc[:, SG + j * P:SG + (j + 1) * P])
            # RoPE from the SBUF copies, deferred into the next PE phase
            def rope_k(G=G, kvc=kvc):
                gsl = slice(G * SG, (G + 1) * SG)
                rope(kvc[:, 0:SG], kT_sb[0:64, gsl], kT_sb[64:128, gsl], G)
            rope_q.append(rope_k)
            for i, (src, lo) in enumerate(((q1c, 0), (q1c, SG),
                                           (q2c, 0), (q2c, SG))):
                dst = qp.tile([P, SG], bf, tag="qT", bufs=8, name="qT")
                qts[(G, i)] = dst
                def rope_qh(src=src, lo=lo, dst=dst, G=G):
                    rope(src[:, lo:lo + SG], dst[0:64, :], dst[64:128, :], G)
                rope_q.append(rope_qh)

        def pass_b(G, drain_ropes=False):
            # order: diagonal tiles first (d0 full-width carries start=True),
            # then full tiles; the LAST tile is full-width (carries stop) --
            # for G0 that's the offset-384 diag with its dead region zeroed.
            diag = [e for e in sk_lists[G] if e[1] is not None]
            plain = [e for e in sk_lists[G] if e[1] is None]
            diag.sort(key=lambda e: e[0])
            # (m, pat, col_off)
            tiles = [(m, pat, pat_offs[pat]) for m, pat in diag] + \
                    [(m, pat, 0) for m, pat in plain]
            n_sk = len(tiles)
            npair = (n_sk + 1) // 2
            g0 = not plain
            for h in range(NH_LOC):
                cacc = ps.tile([P, SG], f32, tag="bank", bufs=2, name="cacc")
                # bufs=4: the reciprocal reading sums(h) is emitted a phase
                # later, so all 4 heads' sums must stay live
                sums = ap_.tile([P, SG], f32, tag="sums", bufs=4)
                # softmax-sum folding on the DVE at [P, 2*SG] granularity:
                # whole ex-pair buffers are added elementwise (u = ex_a+ex_b
                # covers 4 tiles in one op), then u's fold pairwise; gpsimd
                # gets exactly ONE all-reduce per head (its tensor ops are
                # ~4x slower than DVE and a per-pair gpsimd chain stalls
                # the whole pipeline through the rings + strict FIFOs)
                ustack = []   # [tile [P,2SG], level]
                pend_ex = []  # full-width ex awaiting a partner
                part_ts = []  # (restricted t, col offset)

                def fold_push(ex):
                    if not pend_ex:
                        pend_ex.append(ex)
                        return
                    a = pend_ex.pop()
                    u = ap_.tile([P, 2 * SG], bf, tag="tu", bufs=4)
                    nc.vector.tensor_add(u[:], a[:], ex[:])
                    ustack.append([u, 0])
                    while (len(ustack) > 2
                           and ustack[-1][1] == ustack[-2][1]):
                        ub = ustack.pop()
                        ua = ustack.pop()
                        nu = ap_.tile([P, 2 * SG], bf, tag="tu", bufs=4)
                        nc.vector.tensor_add(nu[:], ua[0][:], ub[0][:])
                        ustack.append([nu, max(ua[1], ub[1]) + 1])

                def emit_pair(p):
                    w = 2 if 2 * p + 1 < n_sk else 1
                    pr = tiles[2 * p:2 * p + w]
                    stp = ps.tile([P, 2 * SG], f32, tag="pair", bufs=3,
                                  name="stp")
                    for j, (m, pat, off) in enumerate(pr):
                        nc.tensor.matmul(stp[:, j * SG + off:(j + 1) * SG],
                                         kT_sb[:, m * P:(m + 1) * P],
                                         qts[(G, h)][:, off:SG],
                                         start=True, stop=True)
                    ex = ep.tile([P, 2 * SG], bf, tag="ex", bufs=5)
                    offs = [t[2] for t in pr]
                    if w == 2 and offs[0] == 0 and offs[1] == 0:
                        nc.scalar.activation(ex[:, 0:2 * SG], stp[:, 0:2 * SG],
                                             Exp)
                    else:
                        for j, (m, pat, off) in enumerate(pr):
                            nc.scalar.activation(
                                ex[:, j * SG + off:(j + 1) * SG],
                                stp[:, j * SG + off:(j + 1) * SG], Exp)
                    if g0 and p == npair - 1:
                        # last G0 tile runs its PV full-width to carry the
                        # stop flag; zero its dead region
                        nc.vector.memset(ex[:, SG:SG + offs[1]], 0.0)
                    for j, (m, pat, off) in enumerate(pr):
                        if pat is not None:
                            nc.vector.tensor_mul(
                                ex[:, j * SG + off:(j + 1) * SG],
                                ex[:, j * SG + off:(j + 1) * SG],
                                mb_sb[:, pat * SG + off:(pat + 1) * SG])
                    # softmax-denominator accumulation (no PE involvement)
                    moff = max(offs)
                    if moff:
                        t = ap_.tile([P, SG], bf, tag="tp", bufs=3)
                        if w == 2:
                            nc.vector.tensor_add(t[:, moff:SG],
                                                 ex[:, moff:SG],
                                                 ex[:, SG + moff:2 * SG])
                        else:
                            nc.vector.tensor_copy(t[:, moff:SG],
                                                  ex[:, moff:SG])
                        part_ts.append((t, moff))
                    elif w == 2:
                        fold_push(ex)
                    else:
                        part_ts.append((ex, 0))
                    return ex

                exq = [emit_pair(p) for p in range(min(3, npair))]
                for p in range(npair):
                    ex = exq[p]
                    w = 2 if 2 * p + 1 < n_sk else 1
                    for j, (m, pat, off) in enumerate(tiles[2 * p:2 * p + w]):
                        idx = 2 * p + j
                        if g0 and idx == n_sk - 1:
                            off = 0      # full-width PV carries the stop
                        nc.tensor.matmul(cacc[:, off:SG],
                                         v_sb[:, m * HD:(m + 1) * HD],
                                         ex[:, j * SG + off:(j + 1) * SG],
                                         start=(idx == 0), stop=(idx == n_sk - 1))
                    if p + 3 < npair:
                        exq.append(emit_pair(p + 3))
                # stage the unnormalized context out of PSUM so the bank
                # ring never waits on the softmax-sum chain
                gsl = slice(h * S + G * SG, h * S + (G + 1) * SG)
                nc.vector.tensor_copy(ctx_sb[:, gsl], cacc[:])
                # collapse u-stack + leftovers into one [P, SG] tile
                while len(ustack) > 1:
                    ub = ustack.pop()
                    ua = ustack.pop()
                    nu = ap_.tile([P, 2 * SG], bf, tag="tu", bufs=4)
                    nc.vector.tensor_add(nu[:], ua[0][:], ub[0][:])
                    ustack.append([nu, max(ua[1], ub[1]) + 1])
                th = ap_.tile([P, SG], bf, tag="th", bufs=4)
                if ustack:
                    u = ustack.pop()[0]
                    nc.vector.tensor_add(th[:], u[:, 0:SG], u[:, SG:2 * SG])
                    if pend_ex:
                        a = pend_ex.pop()
                        nc.vector.tensor_add(th[:], th[:], a[:, 0:SG])
                        nc.vector.tensor_add(th[:], th[:], a[:, SG:2 * SG])
                else:
                    a = pend_ex.pop()
                    nc.vector.tensor_add(th[:], a[:, 0:SG], a[:, SG:2 * SG])
                for t, moff in part_ts:
                    nc.vector.tensor_add(th[:, moff:SG], th[:, moff:SG],
                                         t[:, moff:SG])
                nc.gpsimd.partition_all_reduce(sums[:], th[:], P, ReduceOp.add)

                def fin(sums=sums, gsl=gsl):
                    inv = ap_.tile([P, SG], f32, tag="inv", bufs=2)
                    nc.vector.reciprocal_approx_fast(inv[:], sums[:])
                    nc.vector.tensor_mul(ctx_sb[:, gsl], ctx_sb[:, gsl], inv[:])
                fin_q.append(fin)
                if drain_ropes and rope_q:
                    flush(rope_q, 2)

        # ---------------- schedule ----------------
        xs0 = stream_x(0)
        sweep(0, xs0)
        xs1 = stream_x(1)
        load_mb()
        sweep(1, xs1)        # + RoPE(0) interleaved
        xs2 = stream_x(2)
        pass_b(0)
        load_wo()
        sweep(2, xs2)        # + RoPE(1), fins(b0)
        xs3 = stream_x(3)
        pass_b(1)
        sweep(3, xs3)        # + RoPE(2), fins(b1)
        pass_b(2, drain_ropes=True)   # RoPE(3) rides b2's DVE slack
        flush(rope_q)
        pass_b(3)

        # ---------------- pass C: out = ctx @ wo (partial) ----------------
        for m in range(NSK):
            if fin_q:
                flush(fin_q, 1)
            for half2 in range(2):
                orow = cp.tile([P, D // 2], bf, tag="orow", bufs=2)
                for n2 in range(0, D // SG // 2, 2):
                    n = half2 * (D // SG // 2) + n2
                    pop = ps.tile([P, 2 * SG], f32, tag="pair", bufs=3,
                                  name="pop")
                    po = [pop[:, 0:SG], pop[:, SG:2 * SG]]
                    for kk in range(NH_LOC):
                        for i in range(2):
                            nc.tensor.matmul(
                                po[i],
                                ctx_sb[:, kk * S + m * P:kk * S + (m + 1) * P],
                                wo_sb[:, ((n + i) * NH_LOC + kk) * SG:
                                         ((n + i) * NH_LOC + kk + 1) * SG],
                                start=(kk == 0), stop=(kk == NH_LOC - 1))
                    for i in range(2):
                        dst = orow[:, (n2 + i) * SG:(n2 + i + 1) * SG]
                        if i:
                            nc.scalar.copy(dst, po[i])
                        else:
                            nc.vector.tensor_copy(dst, po[i])
                nc.sync.dma_start(
                    out_d[m * P:(m + 1) * P,
                          half2 * (D // 2):(half2 + 1) * (D // 2)],
                    orow[:])

    nc.compile()
    return nc


def _host_prep(x, wq, wk, wv, wo, freqs_cos, freqs_sin):
    """Build per-core input maps (all layouts pre-tiled for contiguous DMA)."""
    from concourse import mybir
    BF = np.dtype(mybir.dt.np(mybir.dt.bfloat16))

    x = np.ascontiguousarray(np.asarray(x, dtype=np.float32).reshape(S, D))
    wq = np.asarray(wq, dtype=np.float32)
    wk = np.asarray(wk, dtype=np.float32)
    wv = np.asarray(wv, dtype=np.float32)
    wo = np.asarray(wo, dtype=np.float32)

    perm = np.concatenate([np.arange(0, HD, 2), np.arange(1, HD, 2)])
    scale = 1.0 / math.sqrt(HD)
    wq_p = (wq.reshape(D, N_HEADS, HD)[:, :, perm] * scale).astype(np.float32)
    wk_p = wk.reshape(D, N_KV, HD)[:, :, perm]

    xtb = np.ascontiguousarray(
        x.T.reshape(KT, P, NG, SG).transpose(1, 2, 0, 3)).astype(BF)
    xtb = np.ascontiguousarray(xtb.reshape(P, NG * KT * SG))
    cosT = np.asarray(freqs_cos, np.float32).T
    sinT = np.asarray(freqs_sin, np.float32).T
    cs = np.ascontiguousarray(
        np.concatenate([np.concatenate([cosT, sinT], axis=0),
                        np.concatenate([sinT, cosT], axis=0)],
                       axis=1)).astype(BF)

    in_maps = []
    for c in range(N_CORES):
        wq_c = wq_p[:, 4 * c:4 * c + 4, :].reshape(D, NH_LOC * HD)
        wq_l = np.ascontiguousarray(
            wq_c.reshape(KT, P, NH_LOC * HD).transpose(1, 0, 2)
            .reshape(P, KT * NH_LOC * HD)).astype(BF)
        wk_c = wk_p[:, c, :]
        wk_l = np.ascontiguousarray(
            wk_c.reshape(KT, P, HD).transpose(1, 0, 2).reshape(P, KT * HD)).astype(BF)
        wv_c = wv.reshape(D, N_KV, HD)[:, c, :]
        wv_l = np.ascontiguousarray(
            wv_c.reshape(KT, P, HD).transpose(1, 0, 2).reshape(P, KT * HD)).astype(BF)
        wo_c = wo[4 * c * HD:(4 * c + 4) * HD, :]       # [512, D]
        wo_l = np.ascontiguousarray(
            wo_c.reshape(NH_LOC, P, D // SG, SG).transpose(1, 2, 0, 3)
            .reshape(P, (D // SG) * NH_LOC * SG)).astype(BF)
        in_maps.append({"xt": xtb, "wq": wq_l, "wk": wk_l,
                        "wv": wv_l, "wo": wo_l, "cs": cs})
    return in_maps


def _run(x, wq, wk, wv, wo, freqs_cos, freqs_sin, mask, start_pos, trace=False):
    assert int(start_pos) == 0
    from concourse import mybir
    BF = np.dtype(mybir.dt.np(mybir.dt.bfloat16))
    sk_lists, patterns = _classify_mask(np.asarray(mask, dtype=np.float32))
    n_pat = len(patterns)
    pat_offs = [_diag_off(p) for p in patterns]
    fp = (tuple(tuple(lst) for lst in sk_lists), n_pat, tuple(pat_offs))

    if fp not in _CACHE:
        _CACHE[fp] = _build_program(sk_lists, n_pat, pat_offs)
    nc = _CACHE[fp]

    in_maps = _host_prep(x, wq, wk, wv, wo, freqs_cos, freqs_sin)
    if n_pat:
        mb = np.ascontiguousarray(np.stack(patterns)).astype(BF)
        for m in in_maps:
            m["mb"] = mb

    from concourse.bass_utils import run_bass_kernel_spmd
    res = run_bass_kernel_spmd(nc, in_maps, list(range(N_CORES)), trace=trace)
    out = np.zeros((S, D), dtype=np.float32)
    for c in range(N_CORES):
        out += res.results[c]["out"].astype(np.float32)
    return out.reshape(1, S, D), res


def kernel(x, wq, wk, wv, wo, freqs_cos, freqs_sin, mask, start_pos):
    out, _ = _run(x, wq, wk, wv, wo, freqs_cos, freqs_sin, mask, start_pos)
    return out


# revision 25
# speedup vs baseline: 1.4626x; 1.0118x over previous
"""Trainium2 Bass kernel for Llama-style GQA attention (B=1, S=2048, D=4096,
32 Q heads / 8 KV heads, head_dim 128, RoPE, additive mask, causal-aware).

Sharding: 8-way tensor-parallel over heads. Core c computes Q heads 4c..4c+3
and KV head c end-to-end; the host sums the 8 partial [S, D] outputs (the
all-reduce of the row-parallel wo).

v4 strategy (over v3): the PE runs ONLY real matmuls -- the softmax
denominator and its broadcast are moved off the PE entirely:
  - Fused per-group sweep: K, V and the 4 Q heads accumulate in one k-loop
    (6 matmuls per k-tile, 3 PSUM pairs: kv + 2 q-pairs).  x is streamed
    once per group and each chunk's buffer frees early (smooth prefetch).
  - Post-sweep, ACT stages the 3 PSUM pairs to SBUF bf16 fast (pair slots
    recycle in ~1us); RoPE runs later from the copies (all-bf16, 2x DVE
    rate) interleaved into the NEXT PE-heavy phase where the DVE is idle.
    V dma-transposes ride the idle sync ring.
  - Softmax sum: DVE adds each exp-pair (bf16), GPSIMD accumulates the
    pair-sums (fp32) and partition_all_reduce broadcasts the key-sum to
    all partitions; DVE reciprocal + in-place normalize of the context.
    The context is staged out of PSUM unnormalized (DVE copy) so the PSUM
    bank ring never waits on the slow gpsimd chain; reciprocal+multiply
    are emitted one phase later (deadline: pass C reads ctx much later).
  - Diagonal mask tiles at offsets 256/384 are computed at half width
    (columns below the offset are fully masked); their exp(mask) patterns
    make full-width reads exact where needed.
  - Schedule: sweep(G+1) is emitted before attention(G) so RoPE/copies
    have a whole phase of slack; pass C (wo) gets 3-deep PSUM pipelining.
"""

import math
import os
import numpy as np

os.environ.setdefault("NEURON_RT_RESET_CORES", "1")

P = 128          # SBUF partitions / head_dim / tile edge
S = 2048         # sequence length
D = 4096         # model dim
HD = 128         # head dim
N_HEADS = 32
N_KV = 8
N_CORES = 8
NH_LOC = N_HEADS // N_CORES   # 4 local Q heads
SG = 512         # score/free-dim group width (one PSUM bank of fp32)
NG = S // SG     # 4 q-position groups
KT = D // P      # 32 contraction tiles for projections
NSK = S // P     # 16 key tiles

_CACHE = {}


def _classify_mask(mask):
    """Classify each [P, SG] block of mask.T into skip / plain / masked."""
    mt = np.ascontiguousarray(mask.T.astype(np.float32))
    patterns = []
    pat_idx = {}
    sk_lists = []
    for G in range(NG):
        lst = []
        for m in range(NSK):
            blk = mt[m * P:(m + 1) * P, G * SG:(G + 1) * SG]
            if np.all(np.isneginf(blk)):
                continue
            if np.all(blk == 0.0):
                lst.append((m, None))
                continue
            with np.errstate(over="ignore"):
                pat = np.exp(blk).astype(np.float32)
            key = pat.tobytes()
            if key not in pat_idx:
                pat_idx[key] = len(patterns)
                patterns.append(pat)
            lst.append((m, pat_idx[key]))
        sk_lists.append(lst)
    return sk_lists, patterns


def _diag_off(pat_np, thresh=256):
    """Column offset below which a pattern block is entirely zero (0 if the
    leading-zero span is < thresh; offsets are quantized to {0, 256})."""
    colmax = pat_np.max(axis=0)
    nz = np.nonzero(colmax)[0]
    first = int(nz[0]) if len(nz) else pat_np.shape[1]
    return 256 if first >= thresh else 0


def _build_program(sk_lists, n_pat, pat_offs):
    import concourse.tile as tile
    from concourse import bacc, mybir
    from concourse.bass_isa import ReduceOp
    from contextlib import ExitStack

    f32 = mybir.dt.float32
    bf = mybir.dt.bfloat16
    Exp = mybir.ActivationFunctionType.Exp

    nc = bacc.Bacc()
    xt_d = nc.dram_tensor("xt", [P, NG * KT * SG], bf, kind="ExternalInput")
    wq_d = nc.dram_tensor("wq", [P, KT * NH_LOC * HD], bf, kind="ExternalInput")
    wk_d = nc.dram_tensor("wk", [P, KT * HD], bf, kind="ExternalInput")
    wv_d = nc.dram_tensor("wv", [P, KT * HD], bf, kind="ExternalInput")
    wo_d = nc.dram_tensor("wo", [P, (D // SG) * NH_LOC * SG], bf,
                          kind="ExternalInput")
    # [cos;sin] in cols [0:S], [sin;cos] in cols [S:2S] -- both partition
    # layouts, because DVE tensor_tensor requires equal base partitions
    # when both inputs are in SBUF
    cs_d = nc.dram_tensor("cs", [P, 2 * S], bf, kind="ExternalInput")
    mb_d = None
    if n_pat:
        mb_d = nc.dram_tensor("mb", [n_pat, P, SG], bf, kind="ExternalInput")
    out_d = nc.dram_tensor("out", [S, D], bf, kind="ExternalOutput")

    XCH = 4 * SG     # xT DMA chunk: 4 k-tiles, 4KB per partition line

    with ExitStack() as ctx:
        tc = ctx.enter_context(tile.TileContext(nc))
        consts = ctx.enter_context(tc.tile_pool(name="consts", bufs=1))
        kv = ctx.enter_context(tc.tile_pool(name="kv", bufs=1))
        xp = ctx.enter_context(tc.tile_pool(name="xp", bufs=7))
        qp = ctx.enter_context(tc.tile_pool(name="qp", bufs=8))
        rp = ctx.enter_context(tc.tile_pool(name="rp", bufs=4))
        ep = ctx.enter_context(tc.tile_pool(name="ep", bufs=5))
        sp = ctx.enter_context(tc.tile_pool(name="sp", bufs=4))
        ap_ = ctx.enter_context(tc.tile_pool(name="ap", bufs=8))
        cp = ctx.enter_context(tc.tile_pool(name="cp", bufs=4))
        ps = ctx.enter_context(tc.tile_pool(name="ps", bufs=8, space="PSUM"))

        # resident weights / constants on the scalar ring; the xT stream and
        # V transposes own the sync ring.
        wk_sb = consts.tile([P, KT * HD], bf)
        wv_sb = consts.tile([P, KT * HD], bf)
        wq_sb = consts.tile([P, KT * NH_LOC * HD], bf)
        # leads so the first k-tiles' matmuls start as early as possible,
        # then supply in k-range order (the fused sweep consumes wk/wv/wq
        # together, so the stream must interleave them, not batch by tensor)
        nc.scalar.dma_start(wk_sb[:, 0:2 * HD], wk_d[:, 0:2 * HD])
        nc.scalar.dma_start(wv_sb[:, 0:2 * HD], wv_d[:, 0:2 * HD])
        wqt = NH_LOC * HD  # per-k-tile wq block
        nc.scalar.dma_start(wq_sb[:, 0:2 * wqt], wq_d[:, 0:2 * wqt])
        qqt = KT * NH_LOC * HD // 8
        for r in range(4):           # k in [8r, 8r+8)
            klo, khi = 8 * r * HD, 8 * (r + 1) * HD
            klo = max(klo, 2 * HD)
            nc.scalar.dma_start(wk_sb[:, klo:khi], wk_d[:, klo:khi])
            nc.scalar.dma_start(wv_sb[:, klo:khi], wv_d[:, klo:khi])
            for i in (2 * r, 2 * r + 1):
                lo = max(i * qqt, 2 * wqt)
                if lo < (i + 1) * qqt:
                    nc.scalar.dma_start(wq_sb[:, lo:(i + 1) * qqt],
                                        wq_d[:, lo:(i + 1) * qqt])
        cs_sb = consts.tile([P, 2 * S], bf)
        nc.scalar.dma_start(cs_sb[:, 0:S], cs_d[:, 0:S])
        nc.scalar.dma_start(cs_sb[:, S:2 * S], cs_d[:, S:2 * S])
        mb_sb = None
        if n_pat:
            mb_sb = consts.tile([P, n_pat * SG], bf, name="mb_sb")
        wo_sb = consts.tile([P, (D // SG) * NH_LOC * SG], bf)

        def load_mb():
            for i in range(n_pat):
                nc.scalar.dma_start(mb_sb[:, i * SG:(i + 1) * SG], mb_d[i])

        def load_wo():
            for i in range(8):
                nc.scalar.dma_start(wo_sb[:, i * qqt:(i + 1) * qqt],
                                    wo_d[:, i * qqt:(i + 1) * qqt])

        # full-sequence KV + context accumulators
        kT_sb = kv.tile([P, S], bf)                  # [head_dim', s]
        v_sb = kv.tile([P, S], bf)                   # [s%P, (s//P)*HD + hd]
        ctx_sb = kv.tile([P, NH_LOC * S], bf)        # [hd, h*S + sq]

        def stream_x(G):
            """DMA the 8 xT chunks of group G; returns per-k slices."""
            slices = []
            for c2 in range(KT * SG // XCH):
                xw = xp.tile([P, XCH], bf, tag="xt", bufs=7, name="xt")
                blk = G * KT * SG + c2 * XCH
                if G == 0 and c2 == 0:
                    nc.sync.dma_start(xw[:, 0:SG], xt_d[:, blk:blk + SG])
                    nc.sync.dma_start(xw[:, SG:XCH],
                                      xt_d[:, blk + SG:blk + XCH])
                else:
                    nc.sync.dma_start(xw[:], xt_d[:, blk:blk + XCH])
                for j in range(XCH // SG):
                    slices.append(xw[:, j * SG:(j + 1) * SG])
            return slices

        def rope(src, dr, di, G):
            cos0 = cs_sb[0:64, G * SG:(G + 1) * SG]
            sin64 = cs_sb[64:128, G * SG:(G + 1) * SG]
            sin0 = cs_sb[0:64, S + G * SG:S + (G + 1) * SG]
            cos64 = cs_sb[64:128, S + G * SG:S + (G + 1) * SG]
            ta = rp.tile([64, SG], bf, tag="ropeA", bufs=2)
            tb = rp.tile([64, SG], bf, tag="ropeB", bufs=2)
            nc.vector.tensor_mul(ta[:], src[0:64, :], cos0)
            nc.vector.tensor_mul(tb[:], src[64:128, :], sin64)
            nc.vector.tensor_sub(dr, ta[:], tb[:])
            tc2 = rp.tile([64, SG], bf, tag="ropeA", bufs=2)
            td = rp.tile([64, SG], bf, tag="ropeB", bufs=2)
            nc.vector.tensor_mul(tc2[:], src[0:64, :], sin0)
            nc.vector.tensor_mul(td[:], src[64:128, :], cos64)
            nc.vector.tensor_add(di, tc2[:], td[:])

        qts = {}
        rope_q = []      # DVE: RoPE of the latest sweep (run in next phase)
        fin_q = []       # DVE: reciprocal + in-place ctx normalize

        def flush(queue, n=None):
            todo = queue[:n] if n is not None else queue[:]
            del queue[:len(todo)]
            for t in todo:
                t()

        def sweep(G, xs):
            """Fused K/V/Q projections for s-group G: 6 matmuls per k-tile."""
            pkv = ps.tile([P, 2 * SG], f32, tag="pair", bufs=3, name="pkv")
            pq1 = ps.tile([P, 2 * SG], f32, tag="pair", bufs=3, name="pq1")
            pq2 = ps.tile([P, 2 * SG], f32, tag="pair", bufs=3, name="pq2")
            dsts = [pkv[:, 0:SG], pkv[:, SG:2 * SG],
                    pq1[:, 0:SG], pq1[:, SG:2 * SG],
                    pq2[:, 0:SG], pq2[:, SG:2 * SG]]
            for k in range(KT):
                st, sp_ = (k == 0), (k == KT - 1)
                nc.tensor.matmul(dsts[0], wk_sb[:, k * HD:(k + 1) * HD],
                                 xs[k], start=st, stop=sp_)
                nc.tensor.matmul(dsts[1], wv_sb[:, k * HD:(k + 1) * HD],
                                 xs[k], start=st, stop=sp_)
                for l in range(NH_LOC):
                    nc.tensor.matmul(
                        dsts[2 + l],
                        wq_sb[:, (k * NH_LOC + l) * HD:(k * NH_LOC + l + 1) * HD],
                        xs[k], start=st, stop=sp_)
                # previous sweep's deferred DVE work rides this PE-heavy loop
                if k % 6 == 5 and rope_q:
                    flush(rope_q, 1)
                if k % 8 == 7 and fin_q:
                    flush(fin_q, 1)
            # stage PSUM -> SBUF bf16 promptly on ACT: pair slots recycle in
            # ~1us each so the next phase's stp allocations never stall long
            kvc = sp.tile([P, 2 * SG], bf, tag="swcp", bufs=3, name="kvc")
            q1c = sp.tile([P, 2 * SG], bf, tag="swcp", bufs=3, name="q1c")
            q2c = sp.tile([P, 2 * SG], bf, tag="swcp", bufs=3, name="q2c")
            nc.scalar.copy(kvc[:], pkv[:])
            nc.scalar.copy(q1c[:], pq1[:])
            nc.scalar.copy(q2c[:], pq2[:])
            # V transposes on the idle sync ring (deadline: attention(G))
            for j in range(SG // P):
                nc.sync.dma_start_transpose(
                    v_sb[:, (G * 4 + j) * HD:(G * 4 + j + 1) * HD],
                    kvc[:, SG + j * P:SG + (j + 1) * P])
            # RoPE from the SBUF copies, deferred into the next PE phase
            def rope_k(G=G, kvc=kvc):
                gsl = slice(G * SG, (G + 1) * SG)
                rope(kvc[:, 0:SG], kT_sb[0:64, gsl], kT_sb[64:128, gsl], G)
            rope_q.append(rope_k)
            for i, (src, lo) in enumerate(((q1c, 0), (q1c, SG),
                                           (q2c, 0), (q2c, SG))):
                dst = qp.tile([P, SG], bf, tag="qT", bufs=8, name="qT")
                qts[(G, i)] = dst
                def rope_qh(src=src, lo=lo, dst=dst, G=G):
                    rope(src[:, lo:lo + SG], dst[0:64, :], dst[64:128, :], G)
                rope_q.append(rope_qh)

        def pass_b(G, drain_ropes=False):
            # order: plain tiles first (no DVE dependency -- each head's
            # first PVs never wait on mask-muls, which ride the lookahead
            # at the tail), diagonal tiles last.  idx0 is full-width and
            # carries start=True; the last tile (offset-384 diag) runs its
            # PV full-width with its dead ex region zeroed so it can carry
            # the stop flag.
            diag = [e for e in sk_lists[G] if e[1] is not None]
            plain = [e for e in sk_lists[G] if e[1] is None]
            diag.sort(key=lambda e: e[0])
            # (m, pat, col_off)
            tiles = [(m, pat, 0) for m, pat in plain] + \
                    [(m, pat, pat_offs[pat]) for m, pat in diag]
            n_sk = len(tiles)
            npair = (n_sk + 1) // 2
            for h in range(NH_LOC):
                cacc = ps.tile([P, SG], f32, tag="bank", bufs=2, name="cacc")
                # bufs=4: the reciprocal reading sums(h) is emitted a phase
                # later, so all 4 heads' sums must stay live
                sums = ap_.tile([P, SG], f32, tag="sums", bufs=4)
                # softmax-sum folding on the DVE at [P, 2*SG] granularity:
                # whole ex-pair buffers are added elementwise (u = ex_a+ex_b
                # covers 4 tiles in one op), then u's fold pairwise; gpsimd
                # gets exactly ONE all-reduce per head (its tensor ops are
                # ~4x slower than DVE and a per-pair gpsimd chain stalls
                # the whole pipeline through the rings + strict FIFOs)
                ustack = []   # [tile [P,2SG], level]
                pend_ex = []  # full-width ex awaiting a partner
                part_ts = []  # (restricted t, col offset)

                def fold_push(ex):
                    if not pend_ex:
                        pend_ex.append(ex)
                        return
                    a = pend_ex.pop()
                    u = ap_.tile([P, 2 * SG], bf, tag="tu", bufs=4)
                    nc.vector.tensor_add(u[:], a[:], ex[:])
                    ustack.append([u, 0])
                    while (len(ustack) > 2
                           and ustack[-1][1] == ustack[-2][1]):
                        ub = ustack.pop()
                        ua = ustack.pop()
                        nu = ap_.tile([P, 2 * SG], bf, tag="tu", bufs=4)
                        nc.vector.tensor_add(nu[:], ua[0][:], ub[0][:])
                        ustack.append([nu, max(ua[1], ub[1]) + 1])

                def emit_pair(p):
                    w = 2 if 2 * p + 1 < n_sk else 1
                    pr = tiles[2 * p:2 * p + w]
                    stp = ps.tile([P, 2 * SG], f32, tag="pair", bufs=3,
                                  name="stp")
                    for j, (m, pat, off) in enumerate(pr):
                        nc.tensor.matmul(stp[:, j * SG + off:(j + 1) * SG],
                                         kT_sb[:, m * P:(m + 1) * P],
                                         qts[(G, h)][:, off:SG],
                                         start=True, stop=True)
                    ex = ep.tile([P, 2 * SG], bf, tag="ex", bufs=5)
                    offs = [t[2] for t in pr]
                    if w == 2 and offs[0] == 0 and offs[1] == 0:
                        nc.scalar.activation(ex[:, 0:2 * SG], stp[:, 0:2 * SG],
                                             Exp)
                    else:
                        for j, (m, pat, off) in enumerate(pr):
                            nc.scalar.activation(
                                ex[:, j * SG + off:(j + 1) * SG],
                                stp[:, j * SG + off:(j + 1) * SG], Exp)
                    if p == npair - 1 and offs[1]:
                        # last tile runs its PV full-width to carry the
                        # stop flag; zero its dead region
                        nc.vector.memset(ex[:, SG:SG + offs[1]], 0.0)
                    for j, (m, pat, off) in enumerate(pr):
                        if pat is not None:
                            nc.vector.tensor_mul(
                                ex[:, j * SG + off:(j + 1) * SG],
                                ex[:, j * SG + off:(j + 1) * SG],
                                mb_sb[:, pat * SG + off:(pat + 1) * SG])
                    # softmax-denominator accumulation (no PE involvement)
                    moff = max(offs)
                    if moff:
                        t = ap_.tile([P, SG], bf, tag="tp", bufs=3)
                        if w == 2:
                            nc.vector.tensor_add(t[:, moff:SG],
                                                 ex[:, moff:SG],
                                                 ex[:, SG + moff:2 * SG])
                        else:
                            nc.vector.tensor_copy(t[:, moff:SG],
                                                  ex[:, moff:SG])
                        part_ts.append((t, moff))
                    elif w == 2:
                        fold_push(ex)
                    else:
                        part_ts.append((ex, 0))
                    return ex

                exq = [emit_pair(p) for p in range(min(3, npair))]
                for p in range(npair):
                    ex = exq[p]
                    w = 2 if 2 * p + 1 < n_sk else 1
                    for j, (m, pat, off) in enumerate(tiles[2 * p:2 * p + w]):
                        idx = 2 * p + j
                        if idx == n_sk - 1:
                            off = 0      # full-width PV carries the stop
                        nc.tensor.matmul(cacc[:, off:SG],
                                         v_sb[:, m * HD:(m + 1) * HD],
                                         ex[:, j * SG + off:(j + 1) * SG],
                                         start=(idx == 0), stop=(idx == n_sk - 1))
                    if p + 3 < npair:
                        exq.append(emit_pair(p + 3))
                # stage the unnormalized context out of PSUM so the bank
                # ring never waits on the softmax-sum chain
                gsl = slice(h * S + G * SG, h * S + (G + 1) * SG)
                nc.vector.tensor_copy(ctx_sb[:, gsl], cacc[:])
                # collapse u-stack + leftovers into one [P, SG] tile
                while len(ustack) > 1:
                    ub = ustack.pop()
                    ua = ustack.pop()
                    nu = ap_.tile([P, 2 * SG], bf, tag="tu", bufs=4)
                    nc.vector.tensor_add(nu[:], ua[0][:], ub[0][:])
                    ustack.append([nu, max(ua[1], ub[1]) + 1])
                th = ap_.tile([P, SG], bf, tag="th", bufs=4)
                if ustack:
                    u = ustack.pop()[0]
                    nc.vector.tensor_add(th[:], u[:, 0:SG], u[:, SG:2 * SG])
                    if pend_ex:
                        a = pend_ex.pop()
                        nc.vector.tensor_add(th[:], th[:], a[:, 0:SG])
                        nc.vector.tensor_add(th[:], th[:], a[:, SG:2 * SG])
                else:
                    a = pend_ex.pop()
                    nc.vector.tensor_add(th[:], a[:, 0:SG], a[:, SG:2 * SG])
                for t, moff in part_ts:
                    nc.vector.tensor_add(th[:, moff:SG], th[:, moff:SG],
                                         t[:, moff:SG])
                nc.gpsimd.partition_all_reduce(sums[:], th[:], P, ReduceOp.add)

                def fin(sums=sums, gsl=gsl):
                    inv = ap_.tile([P, SG], f32, tag="inv", bufs=2)
                    nc.vector.reciprocal_approx_fast(inv[:], sums[:])
                    nc.vector.tensor_mul(ctx_sb[:, gsl], ctx_sb[:, gsl], inv[:])
                fin_q.append(fin)
                if drain_ropes and rope_q:
                    flush(rope_q, 2)

        # ---------------- schedule ----------------
        xs0 = stream_x(0)
        sweep(0, xs0)
        xs1 = stream_x(1)
        load_mb()
        sweep(1, xs1)        # + RoPE(0) interleaved
        xs2 = stream_x(2)
        pass_b(0)
        load_wo()
        sweep(2, xs2)        # + RoPE(1), fins(b0)
        xs3 = stream_x(3)
        pass_b(1)
        sweep(3, xs3)        # + RoPE(2), fins(b1)
        pass_b(2, drain_ropes=True)   # RoPE(3) rides b2's DVE slack
        flush(rope_q)
        pass_b(3)

        # ---------------- pass C: out = ctx @ wo (partial) ----------------
        for m in range(NSK):
            if fin_q:
                flush(fin_q, 1)
            for half2 in range(2):
                orow = cp.tile([P, D // 2], bf, tag="orow", bufs=2)
                for n2 in range(0, D // SG // 2, 2):
                    n = half2 * (D // SG // 2) + n2
                    pop = ps.tile([P, 2 * SG], f32, tag="pair", bufs=3,
                                  name="pop")
                    po = [pop[:, 0:SG], pop[:, SG:2 * SG]]
                    for kk in range(NH_LOC):
                        for i in range(2):
                            nc.tensor.matmul(
                                po[i],
                                ctx_sb[:, kk * S + m * P:kk * S + (m + 1) * P],
                                wo_sb[:, ((n + i) * NH_LOC + kk) * SG:
                                         ((n + i) * NH_LOC + kk + 1) * SG],
                                start=(kk == 0), stop=(kk == NH_LOC - 1))
                    for i in range(2):
                        dst = orow[:, (n2 + i) * SG:(n2 + i + 1) * SG]
                        if i:
                            nc.scalar.copy(dst, po[i])
                        else:
                            nc.vector.tensor_copy(dst, po[i])
                nc.sync.dma_start(
                    out_d[m * P:(m + 1) * P,
                          half2 * (D // 2):(half2 + 1) * (D // 2)],
                    orow[:])

    nc.compile()
    return nc


def _host_prep(x, wq, wk, wv, wo, freqs_cos, freqs_sin):
    """Build per-core input maps (all layouts pre-tiled for contiguous DMA)."""
    from concourse import mybir
    BF = np.dtype(mybir.dt.np(mybir.dt.bfloat16))

    x = np.ascontiguousarray(np.asarray(x, dtype=np.float32).reshape(S, D))
    wq = np.asarray(wq, dtype=np.float32)
    wk = np.asarray(wk, dtype=np.float32)
    wv = np.asarray(wv, dtype=np.float32)
    wo = np.asarray(wo, dtype=np.float32)

    perm = np.concatenate([np.arange(0, HD, 2), np.arange(1, HD, 2)])
    scale = 1.0 / math.sqrt(HD)
    wq_p = (wq.reshape(D, N_HEADS, HD)[:, :, perm] * scale).astype(np.float32)
    wk_p = wk.reshape(D, N_KV, HD)[:, :, perm]

    xtb = np.ascontiguousarray(
        x.T.reshape(KT, P, NG, SG).transpose(1, 2, 0, 3)).astype(BF)
    xtb = np.ascontiguousarray(xtb.reshape(P, NG * KT * SG))
    cosT = np.asarray(freqs_cos, np.float32).T
    sinT = np.asarray(freqs_sin, np.float32).T
    cs = np.ascontiguousarray(
        np.concatenate([np.concatenate([cosT, sinT], axis=0),
                        np.concatenate([sinT, cosT], axis=0)],
                       axis=1)).astype(BF)

    in_maps = []
    for c in range(N_CORES):
        wq_c = wq_p[:, 4 * c:4 * c + 4, :].reshape(D, NH_LOC * HD)
        wq_l = np.ascontiguousarray(
            wq_c.reshape(KT, P, NH_LOC * HD).transpose(1, 0, 2)
            .reshape(P, KT * NH_LOC * HD)).astype(BF)
        wk_c = wk_p[:, c, :]
        wk_l = np.ascontiguousarray(
            wk_c.reshape(KT, P, HD).transpose(1, 0, 2).reshape(P, KT * HD)).astype(BF)
        wv_c = wv.reshape(D, N_KV, HD)[:, c, :]
        wv_l = np.ascontiguousarray(
            wv_c.reshape(KT, P, HD).transpose(1, 0, 2).reshape(P, KT * HD)).astype(BF)
        wo_c = wo[4 * c * HD:(4 * c + 4) * HD, :]       # [512, D]
        wo_l = np.ascontiguousarray(
            wo_c.reshape(NH_LOC, P, D // SG, SG).transpose(1, 2, 0, 3)
            .reshape(P, (D // SG) * NH_LOC * SG)).astype(BF)
        in_maps.append({"xt": xtb, "wq": wq_l, "wk": wk_l,
                        "wv": wv_l, "wo": wo_l, "cs": cs})
    return in_maps


def _run(x, wq, wk, wv, wo, freqs_cos, freqs_sin, mask, start_pos, trace=False):
    assert int(start_pos) == 0
    from concourse import mybir
    BF = np.dtype(mybir.dt.np(mybir.dt.bfloat16))
    sk_lists, patterns = _classify_mask(np.asarray(mask, dtype=np.float32))
    n_pat = len(patterns)
    pat_offs = [_diag_off(p) for p in patterns]
    fp = (tuple(tuple(lst) for lst in sk_lists), n_pat, tuple(pat_offs))

    if fp not in _CACHE:
        _CACHE[fp] = _build_program(sk_lists, n_pat, pat_offs)
    nc = _CACHE[fp]

    in_maps = _host_prep(x, wq, wk, wv, wo, freqs_cos, freqs_sin)
    if n_pat:
        mb = np.ascontiguousarray(np.stack(patterns)).astype(BF)
        for m in in_maps:
            m["mb"] = mb

    from concourse.bass_utils import run_bass_kernel_spmd
    res = run_bass_kernel_spmd(nc, in_maps, list(range(N_CORES)), trace=trace)
    out = np.zeros((S, D), dtype=np.float32)
    for c in range(N_CORES):
        out += res.results[c]["out"].astype(np.float32)
    return out.reshape(1, S, D), res


def kernel(x, wq, wk, wv, wo, freqs_cos, freqs_sin, mask, start_pos):
    out, _ = _run(x, wq, wk, wv, wo, freqs_cos, freqs_sin, mask, start_pos)
    return out
